# revision 15
# baseline (speedup 1.0000x reference)
"""Trainium2 Bass kernel: C = Au @ Bu for packed upper-triangular Au, Bu.

Inputs (full): A, B — packed row-major upper-triangular storage of two
512x512 f32 matrices, each a flat array of length 131328 = 512*513/2.
Output: dense [512, 512] f32 C = unpack(A) @ unpack(B)  (upper triangular).

Strategy:
  - Host-side layout prep (pure O(N^2) data movement): unpack packed ->
    dense, form A^T (lhsT layout) and B, slice per core.
  - 4x2 core grid: core g = (R, c) computes the C block
    [128R:128R+128, 256c:256c+256] — full 128 output partitions, free
    dim 256 per matmul.
  - Raw bacc program (no Tile scheduling ceremony): one combined input
    tensor per core split into 2 k-chunk DMAs (staggered completion),
    4 accumulating PE matmuls into one PSUM bank, one DVE copy, one
    output DMA. Entry const-AP memsets stripped (they gate the entry
    barrier on the Pool engine).
"""

import numpy as np

N = 512
P = 128
KT = 4
NCORES = 8
GRID = (4, 2)  # (row bands, col bands)
MB = N // GRID[0]  # 128 rows of C per core
NB = N // GRID[1]  # 256 cols of C per core
W = MB + NB
IN_SPLIT = 2
DTYPE = "f32r"  # "bf16" | "f32" | "f32r"
PACKED_LEN = N * (N + 1) // 2

_CACHE = {}


def _unpack_upper(p):
    """Packed row-major upper-tri -> dense [N, N] with zero lower triangle."""
    p = np.asarray(p, dtype=np.float32).reshape(-1)
    i = np.arange(N)[:, None]
    j = np.arange(N)[None, :]
    mask = j >= i
    pidx = np.where(mask, (i * (2 * N - i + 1)) // 2 + (j - i), 0)
    return np.where(mask, p[pidx], np.float32(0.0))


def _store_np_dtype():
    if DTYPE == "bf16":
        import ml_dtypes

        return ml_dtypes.bfloat16
    return np.float32


def _strip_const_memsets(nc):
    """Remove the 4 unused const-AP memsets from the entry block (they gate
    the entry all-engine barrier on the Pool engine by ~400ns)."""
    import concourse.mybir as mybir

    bb = nc.m.functions[0].blocks[0]
    bb.instructions = [
        i
        for i in bb.instructions
        if not (
            isinstance(i, mybir.InstMemset)
            and i.outs
            and "const-" in str(getattr(i.outs[0].bass_ap.tensor, "name", ""))
        )
    ]


def _build_nc():
    import concourse.mybir as mybir
    from concourse import bacc

    F32 = mybir.dt.float32
    store_dt = {
        "bf16": mybir.dt.bfloat16,
        "f32": F32,
        "f32r": mybir.dt.float32r,
    }[DTYPE]
    per = KT // IN_SPLIT

    nc = bacc.Bacc("TRN2", num_devices=NCORES)
    ab = nc.dram_tensor("ab", [P, KT, W], store_dt, kind="ExternalInput")
    cdr = nc.dram_tensor("c", [MB, NB], F32, kind="ExternalOutput")

    with (
        nc.sbuf_tensor([P, KT, W], store_dt) as t,
        nc.sbuf_tensor([MB, NB], F32) as ostage,
        nc.psum_tensor([MB, NB], F32) as psum,
        nc.semaphore("dsem0") as dsem0,
        nc.semaphore("dsem1") as dsem1,
        nc.semaphore("osem") as osem,
        nc.semaphore("psem") as psem,
        nc.semaphore("vsem") as vsem,
        nc.Block(no_gpsimd_drain=True) as block,
    ):
        # One semaphore per input chunk: two DMAs sharing one sem would let
        # per-engine completions interleave, so a wait at 16 wouldn't prove
        # chunk 0 fully landed (CoreSim race detector flags this).
        dsems = [dsem0, dsem1]
        assert IN_SPLIT == len(dsems)

        @block.sync
        def _(sync):
            for d in range(IN_SPLIT):
                sync.dma_start(
                    out=t.ap()[:, d * per : (d + 1) * per],
                    in_=ab.ap()[:, d * per : (d + 1) * per],
                ).then_inc(dsems[d], 16)
            sync.wait_ge(vsem, 1)
            sync.dma_start(out=cdr.ap(), in_=ostage.ap()).then_inc(osem, 16)
            sync.wait_ge(osem, 16)

        @block.tensor
        def _(tensor):
            last = None
            for kt in range(KT):
                if kt % per == 0:
                    tensor.wait_ge(dsems[kt // per], 16)
                last = nc.tensor.matmul(
                    psum.ap(),
                    t.ap()[:, kt, :MB],
                    t.ap()[:, kt, MB:],
                    start=(kt == 0),
                    stop=(kt == KT - 1),
                )
            last.then_inc(psem, 1)

        @block.vector
        def _(vector):
            vector.wait_ge(psem, 1)
            nc.vector.tensor_copy(ostage.ap(), psum.ap()).then_inc(vsem, 1)

    _strip_const_memsets(nc)
    nc.compile()
    return nc


def _get_nc():
    if "nc" not in _CACHE:
        _CACHE["nc"] = _build_nc()
    return _CACHE["nc"]


def _make_in_maps(A, B):
    Au = _unpack_upper(A)
    Bu = _unpack_upper(B)
    aT = np.ascontiguousarray(Au.T)  # aT[k, m] = Au[m, k]
    sdt = _store_np_dtype()
    aTk = aT.reshape(KT, P, N)  # [kt, p, m]
    Buk = Bu.reshape(KT, P, N)  # [kt, p, n]
    in_maps = []
    for g in range(NCORES):
        R, c = divmod(g, GRID[1])
        abarr = np.empty((P, KT, W), dtype=np.float32)
        # abarr[p, kt, :MB] = aT[kt*128+p, R*MB + m'] ; [.., MB:] = Bu[kt*128+p, c*NB + n']
        abarr[:, :, :MB] = aTk[:, :, R * MB : (R + 1) * MB].transpose(1, 0, 2)
        abarr[:, :, MB:] = Buk[:, :, c * NB : (c + 1) * NB].transpose(1, 0, 2)
        in_maps.append({"ab": abarr.astype(sdt)})
    return in_maps


def _get_runner():
    """Build the sharded PJRT executable once; reuse across kernel() calls.

    Mirrors concourse.bass2jax.run_bass_via_pjrt's multi-core path, but
    caches the jitted function so repeat calls skip retracing.
    """
    if "runner" in _CACHE:
        return _CACHE["runner"]
    import jax
    import concourse.mybir as mybir
    from concourse import bass2jax
    from jax.experimental.shard_map import shard_map
    from jax.sharding import Mesh, PartitionSpec

    nc = _get_nc()
    bass2jax.install_neuronx_cc_hook()
    partition_name = (
        nc.partition_id_tensor.name if nc.partition_id_tensor else None
    )
    in_names, out_names, out_avals, zero_outs = [], [], [], []
    for alloc in nc.m.functions[0].allocations:
        if not isinstance(alloc, mybir.MemoryLocationSet):
            continue
        name = alloc.memorylocations[0].name
        if alloc.kind == "ExternalInput":
            if name != partition_name:
                in_names.append(name)
        elif alloc.kind == "ExternalOutput":
            out_names.append(name)
            shape = tuple(alloc.tensor_shape)
            dtype = mybir.dt.np(alloc.dtype)
            out_avals.append(jax.core.ShapedArray(shape, dtype))
            zero_outs.append(np.zeros(shape, dtype))
    n_params = len(in_names)
    n_outs = len(out_names)
    all_in = in_names + out_names + ([partition_name] if partition_name else [])
    donate = tuple(range(n_params, n_params + n_outs))

    def _body(*args):
        operands = list(args)
        if partition_name is not None:
            operands.append(bass2jax.partition_id_tensor())
        outs = bass2jax._bass_exec_p.bind(
            *operands,
            out_avals=tuple(out_avals),
            in_names=tuple(all_in),
            out_names=tuple(out_names),
            lowering_input_output_aliases=(),
            sim_require_finite=True,
            sim_require_nnan=True,
            nc=nc,
        )
        return tuple(outs)

    devices = jax.devices()[:NCORES]
    mesh = Mesh(np.asarray(devices), ("core",))
    fn = jax.jit(
        shard_map(
            _body,
            mesh=mesh,
            in_specs=(PartitionSpec("core"),) * (n_params + n_outs),
            out_specs=(PartitionSpec("core"),) * n_outs,
            check_rep=False,
        ),
        donate_argnums=donate,
        keep_unused=True,
    )
    runner = dict(
        fn=fn, in_names=in_names, out_names=out_names, zero_outs=zero_outs
    )
    _CACHE["runner"] = runner
    return runner


def _run_concat(concat_in):
    """Execute on 8 cores given axis-0-concatenated per-core inputs."""
    r = _get_runner()
    concat_zeros = [
        np.zeros((NCORES * z.shape[0], *z.shape[1:]), z.dtype)
        for z in r["zero_outs"]
    ]
    return r["fn"](*concat_in, *concat_zeros)


def _concat_inputs(in_maps):
    r = _get_runner()
    return [
        np.concatenate([in_maps[c][n] for c in range(NCORES)], axis=0)
        for n in r["in_names"]
    ]


def _assemble(out0):
    blocks = np.asarray(out0).reshape(NCORES, MB, NB)
    C = np.empty((N, N), dtype=np.float32)
    for g in range(NCORES):
        R, c = divmod(g, GRID[1])
        C[R * MB : (R + 1) * MB, c * NB : (c + 1) * NB] = blocks[g]
    return C


def kernel(A, B):
    in_maps = _make_in_maps(A, B)
    concat_in = _concat_inputs(in_maps)
    out = _run_concat(concat_in)
    return _assemble(out[0])


# revision 16
# speedup vs baseline: 1.6960x; 1.6960x over previous
"""Trainium2 Bass kernel: C = Au @ Bu for packed upper-triangular Au, Bu.

Inputs (full): A, B — packed row-major upper-triangular storage of two
512x512 f32 matrices, each a flat array of length 131328 = 512*513/2.
Output: dense [512, 512] f32 C = unpack(A) @ unpack(B)  (upper triangular).

Strategy — balanced triangular decomposition over a 4x2 C-block grid:
  C is tiled into 8 blocks of [128, 256].  Block (R, c) only needs
  contraction k in [128R, 256(c+1)) because A/B are upper triangular, so
  of the 32 (block, k-block) products only 13 are nonzero.  Those 13 MM
  units are spread over the 8 cores (<=2 each, padded with zero slabs),
  each unit = one 128-deep PE matmul [128k x 128m] @ [128k x 256n]
  accumulating in PSUM.  Blocks (2,0) and (3,0) are structurally zero;
  blocks (0,1) and (1,1) are split across two cores and summed on host.

  Per core: 384KB f32 in (vs 1.15MB for the dense column-shard), 2
  native fp32 matmuls (exact vs the f32 reference up to partial-sum
  ordering), one DVE PSUM->SBUF copy, one 128KB output DMA.  Raw bacc
  program (no Tile ceremony); entry const-AP memsets stripped since they
  gate the entry all-engine barrier on the Pool engine.
"""

import numpy as np

N = 512
P = 128
KT = 4  # k-blocks in the full problem
NCORES = 8
MB = 128  # C-block rows
NB = 256  # C-block cols
UW = MB + NB  # one unit slab: A part [128,128] + B part [128,256]
DTYPE = "f32"  # "bf16" | "f32" | "f32r"
PACKED_LEN = N * (N + 1) // 2

# core -> (unit0, unit1); unit = (R, c, kt) C-block row-band/col-band/k-block,
# or None for a zero-padded slot.
ASSIGN = [
    ((0, 0, 0), (0, 0, 1)),
    ((0, 1, 0), (0, 1, 1)),
    ((0, 1, 2), (0, 1, 3)),
    ((1, 1, 1), (1, 1, 2)),
    ((1, 1, 3), None),
    ((2, 1, 2), (2, 1, 3)),
    ((1, 0, 1), None),
    ((3, 1, 3), None),
]
# C block (R, c) -> list of cores whose outputs sum to it.
BLOCK_CORES = {}
for _g, _units in enumerate(ASSIGN):
    for _u in _units:
        if _u is not None:
            BLOCK_CORES.setdefault((_u[0], _u[1]), []).append(_g)
BLOCK_CORES = {k: sorted(set(v)) for k, v in BLOCK_CORES.items()}

_CACHE = {}


def _unpack_upper(p):
    """Packed row-major upper-tri -> dense [N, N] with zero lower triangle."""
    p = np.asarray(p, dtype=np.float32).reshape(-1)
    i = np.arange(N)[:, None]
    j = np.arange(N)[None, :]
    mask = j >= i
    pidx = np.where(mask, (i * (2 * N - i + 1)) // 2 + (j - i), 0)
    return np.where(mask, p[pidx], np.float32(0.0))


def _store_np_dtype():
    if DTYPE == "bf16":
        import ml_dtypes

        return ml_dtypes.bfloat16
    return np.float32


def _strip_const_memsets(nc):
    """Remove the 4 unused const-AP memsets from the entry block (they gate
    the entry all-engine barrier on the Pool engine by ~400ns)."""
    import concourse.mybir as mybir

    bb = nc.m.functions[0].blocks[0]
    bb.instructions = [
        i
        for i in bb.instructions
        if not (
            isinstance(i, mybir.InstMemset)
            and i.outs
            and "const-" in str(getattr(i.outs[0].bass_ap.tensor, "name", ""))
        )
    ]


def _build_nc():
    import concourse.mybir as mybir
    from concourse import bacc

    F32 = mybir.dt.float32
    store_dt = {
        "bf16": mybir.dt.bfloat16,
        "f32": F32,
        "f32r": mybir.dt.float32r,
    }[DTYPE]

    nc = bacc.Bacc("TRN2", num_devices=NCORES)
    ab = nc.dram_tensor("ab", [P, 2, UW], store_dt, kind="ExternalInput")
    cdr = nc.dram_tensor("c", [MB, NB], F32, kind="ExternalOutput")

    with (
        nc.sbuf_tensor([P, 2, UW], store_dt) as t,
        nc.sbuf_tensor([MB, NB], F32) as ostage,
        nc.psum_tensor([MB, NB], F32) as psum,
        nc.semaphore("dsem0") as dsem0,
        nc.semaphore("dsem1") as dsem1,
        nc.semaphore("osem") as osem,
        nc.semaphore("psem") as psem,
        nc.semaphore("vsem") as vsem,
        nc.Block(no_gpsimd_drain=True) as block,
    ):
        # One semaphore per input chunk: two DMAs sharing one sem could
        # interleave per-engine completions, so a wait at 16 wouldn't prove
        # chunk 0 fully landed (CoreSim race detector flags this).
        dsems = [dsem0, dsem1]

        @block.sync
        def _(sync):
            for u in range(2):
                sync.dma_start(out=t.ap()[:, u], in_=ab.ap()[:, u]).then_inc(
                    dsems[u], 16
                )
            sync.wait_ge(vsem, 1)
            sync.dma_start(out=cdr.ap(), in_=ostage.ap()).then_inc(osem, 16)
            sync.wait_ge(osem, 16)

        @block.tensor
        def _(tensor):
            last = None
            for u in range(2):
                tensor.wait_ge(dsems[u], 16)
                last = nc.tensor.matmul(
                    psum.ap(),
                    t.ap()[:, u, :MB],
                    t.ap()[:, u, MB:],
                    start=(u == 0),
                    stop=(u == 1),
                )
            last.then_inc(psem, 1)

        @block.vector
        def _(vector):
            vector.wait_ge(psem, 1)
            nc.vector.tensor_copy(ostage.ap(), psum.ap()).then_inc(vsem, 1)

    _strip_const_memsets(nc)
    nc.compile()
    return nc


def _get_nc():
    if "nc" not in _CACHE:
        _CACHE["nc"] = _build_nc()
    return _CACHE["nc"]


def _make_in_maps(A, B):
    Au = _unpack_upper(A)
    Bu = _unpack_upper(B)
    aT = np.ascontiguousarray(Au.T)  # aT[k, m] = Au[m, k]
    sdt = _store_np_dtype()
    aTk = aT.reshape(KT, P, N)  # [kt, p, m]
    Buk = Bu.reshape(KT, P, N)  # [kt, p, n]
    in_maps = []
    for units in ASSIGN:
        abarr = np.zeros((P, 2, UW), dtype=np.float32)
        for u, unit in enumerate(units):
            if unit is None:
                continue
            R, c, kt = unit
            abarr[:, u, :MB] = aTk[kt, :, R * MB : (R + 1) * MB]
            abarr[:, u, MB:] = Buk[kt, :, c * NB : (c + 1) * NB]
        in_maps.append({"ab": abarr.astype(sdt)})
    return in_maps


def _get_runner():
    """Build the sharded PJRT executable once; reuse across kernel() calls.

    Mirrors concourse.bass2jax.run_bass_via_pjrt's multi-core path, but
    caches the jitted function so repeat calls skip retracing.
    """
    if "runner" in _CACHE:
        return _CACHE["runner"]
    import jax
    import concourse.mybir as mybir
    from concourse import bass2jax
    from jax.experimental.shard_map import shard_map
    from jax.sharding import Mesh, PartitionSpec

    nc = _get_nc()
    bass2jax.install_neuronx_cc_hook()
    partition_name = (
        nc.partition_id_tensor.name if nc.partition_id_tensor else None
    )
    in_names, out_names, out_avals, zero_outs = [], [], [], []
    for alloc in nc.m.functions[0].allocations:
        if not isinstance(alloc, mybir.MemoryLocationSet):
            continue
        name = alloc.memorylocations[0].name
        if alloc.kind == "ExternalInput":
            if name != partition_name:
                in_names.append(name)
        elif alloc.kind == "ExternalOutput":
            out_names.append(name)
            shape = tuple(alloc.tensor_shape)
            dtype = mybir.dt.np(alloc.dtype)
            out_avals.append(jax.core.ShapedArray(shape, dtype))
            zero_outs.append(np.zeros(shape, dtype))
    n_params = len(in_names)
    n_outs = len(out_names)
    all_in = in_names + out_names + ([partition_name] if partition_name else [])
    donate = tuple(range(n_params, n_params + n_outs))

    def _body(*args):
        operands = list(args)
        if partition_name is not None:
            operands.append(bass2jax.partition_id_tensor())
        outs = bass2jax._bass_exec_p.bind(
            *operands,
            out_avals=tuple(out_avals),
            in_names=tuple(all_in),
            out_names=tuple(out_names),
            lowering_input_output_aliases=(),
            sim_require_finite=True,
            sim_require_nnan=True,
            nc=nc,
        )
        return tuple(outs)

    devices = jax.devices()[:NCORES]
    mesh = Mesh(np.asarray(devices), ("core",))
    fn = jax.jit(
        shard_map(
            _body,
            mesh=mesh,
            in_specs=(PartitionSpec("core"),) * (n_params + n_outs),
            out_specs=(PartitionSpec("core"),) * n_outs,
            check_rep=False,
        ),
        donate_argnums=donate,
        keep_unused=True,
    )
    runner = dict(
        fn=fn, in_names=in_names, out_names=out_names, zero_outs=zero_outs
    )
    _CACHE["runner"] = runner
    return runner


def _run_concat(concat_in):
    """Execute on 8 cores given axis-0-concatenated per-core inputs."""
    r = _get_runner()
    concat_zeros = [
        np.zeros((NCORES * z.shape[0], *z.shape[1:]), z.dtype)
        for z in r["zero_outs"]
    ]
    return r["fn"](*concat_in, *concat_zeros)


def _concat_inputs(in_maps):
    r = _get_runner()
    return [
        np.concatenate([in_maps[c][n] for c in range(NCORES)], axis=0)
        for n in r["in_names"]
    ]


def _assemble(out0):
    blocks = np.asarray(out0).reshape(NCORES, MB, NB)
    C = np.zeros((N, N), dtype=np.float32)
    for (R, c), cores in BLOCK_CORES.items():
        acc = blocks[cores[0]].copy()
        for g in cores[1:]:
            acc += blocks[g]
        C[R * MB : (R + 1) * MB, c * NB : (c + 1) * NB] = acc
    return C


def kernel(A, B):
    in_maps = _make_in_maps(A, B)
    concat_in = _concat_inputs(in_maps)
    out = _run_concat(concat_in)
    return _assemble(out[0])


# revision 17
# speedup vs baseline: 15457.4551x; 9114.0445x over previous
"""Trainium2 Bass kernel: C = Au @ Bu for packed upper-triangular Au, Bu.

Inputs (full): A, B — packed row-major upper-triangular storage of two
512x512 f32 matrices, each a flat array of length 131328 = 512*513/2.
Output: dense [512, 512] f32 C = unpack(A) @ unpack(B)  (upper triangular).

Strategy — balanced triangular decomposition over a 4x2 C-block grid:
  C is tiled into 8 blocks of [128, 256].  Block (R, c) only needs
  contraction k in [128R, 256(c+1)) because A/B are upper triangular, so
  of the 32 (block, k-block) products only 13 are nonzero.  Those 13 MM
  units are spread over the 8 cores (<=2 each, padded with zero slabs),
  each unit = one 128-deep PE matmul [128k x 128m] @ [128k x 256n]
  accumulating in PSUM.  Blocks (2,0) and (3,0) are structurally zero;
  blocks (0,1) and (1,1) are split across two cores and summed on host.

  Per core: 384KB f32 in (vs 1.15MB for the dense column-shard), 2
  native fp32 matmuls (exact vs the f32 reference up to partial-sum
  ordering), one DVE PSUM->SBUF copy, one 128KB output DMA.  Raw bacc
  program (no Tile ceremony); entry const-AP memsets stripped since they
  gate the entry all-engine barrier on the Pool engine.
"""

import numpy as np

N = 512
P = 128
KT = 4  # k-blocks in the full problem
NCORES = 8
MB = 128  # C-block rows
NB = 256  # C-block cols
UW = MB + NB  # one unit slab: A part [128,128] + B part [128,256]
DTYPE = "f32"  # "bf16" | "f32" | "f32r"
PACKED_LEN = N * (N + 1) // 2

# core -> (unit0, unit1); unit = (R, c, kt) C-block row-band/col-band/k-block,
# or None for a zero-padded slot.
ASSIGN = [
    ((0, 0, 0), (0, 0, 1)),
    ((0, 1, 0), (0, 1, 1)),
    ((0, 1, 2), (0, 1, 3)),
    ((1, 1, 1), (1, 1, 2)),
    ((1, 1, 3), None),
    ((2, 1, 2), (2, 1, 3)),
    ((1, 0, 1), None),
    ((3, 1, 3), None),
]
# C block (R, c) -> list of cores whose outputs sum to it.
BLOCK_CORES = {}
for _g, _units in enumerate(ASSIGN):
    for _u in _units:
        if _u is not None:
            BLOCK_CORES.setdefault((_u[0], _u[1]), []).append(_g)
BLOCK_CORES = {k: sorted(set(v)) for k, v in BLOCK_CORES.items()}

_CACHE = {}


def _unpack_upper(p):
    """Packed row-major upper-tri -> dense [N, N] with zero lower triangle."""
    p = np.asarray(p, dtype=np.float32).reshape(-1)
    i = np.arange(N)[:, None]
    j = np.arange(N)[None, :]
    mask = j >= i
    pidx = np.where(mask, (i * (2 * N - i + 1)) // 2 + (j - i), 0)
    return np.where(mask, p[pidx], np.float32(0.0))


def _store_np_dtype():
    if DTYPE == "bf16":
        import ml_dtypes

        return ml_dtypes.bfloat16
    return np.float32


def _strip_const_memsets(nc):
    """Remove framework ceremony that isn't needed by this kernel:
    - the 4 unused const-AP memsets in the entry block (they gate the
      entry all-engine barrier on the Pool engine by ~400ns), and
    - the exit all-engine barrier EventSemaphores (the final SP
      wait_ge(osem) already guarantees the output landed; per-engine
      drains are kept)."""
    import concourse.mybir as mybir

    bb = nc.m.functions[0].blocks[0]
    bb.instructions = [
        i
        for i in bb.instructions
        if not (
            isinstance(i, mybir.InstMemset)
            and i.outs
            and "const-" in str(getattr(i.outs[0].bass_ap.tensor, "name", ""))
        )
    ]
    for bb in nc.m.functions[0].blocks:
        if bb.name.endswith("_end"):
            bb.instructions = [
                i
                for i in bb.instructions
                if not (
                    isinstance(i, mybir.InstEventSemaphore)
                    and str(i.name).startswith("aeb_barrier")
                )
            ]


def _build_nc():
    import concourse.mybir as mybir
    from concourse import bacc

    F32 = mybir.dt.float32
    store_dt = {
        "bf16": mybir.dt.bfloat16,
        "f32": F32,
        "f32r": mybir.dt.float32r,
    }[DTYPE]

    nc = bacc.Bacc("TRN2", num_devices=NCORES)
    ab = nc.dram_tensor("ab", [P, 2, UW], store_dt, kind="ExternalInput")
    cdr = nc.dram_tensor("c", [MB, NB], F32, kind="ExternalOutput")

    with (
        nc.sbuf_tensor([P, 2, UW], store_dt) as t,
        nc.sbuf_tensor([MB, NB], F32) as ostage,
        nc.psum_tensor([MB, NB], F32) as psum,
        nc.semaphore("dsem0") as dsem0,
        nc.semaphore("dsem1") as dsem1,
        nc.semaphore("osem") as osem,
        nc.semaphore("psem") as psem,
        nc.semaphore("vsem") as vsem,
        nc.Block(no_gpsimd_drain=True) as block,
    ):
        # One semaphore per input chunk: two DMAs sharing one sem could
        # interleave per-engine completions, so a wait at 16 wouldn't prove
        # chunk 0 fully landed (CoreSim race detector flags this).
        dsems = [dsem0, dsem1]

        @block.sync
        def _(sync):
            for u in range(2):
                sync.dma_start(out=t.ap()[:, u], in_=ab.ap()[:, u]).then_inc(
                    dsems[u], 16
                )
            sync.wait_ge(vsem, 1)
            sync.dma_start(out=cdr.ap(), in_=ostage.ap()).then_inc(osem, 16)
            sync.wait_ge(osem, 16)

        @block.tensor
        def _(tensor):
            last = None
            for u in range(2):
                tensor.wait_ge(dsems[u], 16)
                last = nc.tensor.matmul(
                    psum.ap(),
                    t.ap()[:, u, :MB],
                    t.ap()[:, u, MB:],
                    start=(u == 0),
                    stop=(u == 1),
                )
            last.then_inc(psem, 1)

        @block.vector
        def _(vector):
            vector.wait_ge(psem, 1)
            nc.vector.tensor_copy(ostage.ap(), psum.ap()).then_inc(vsem, 1)

    _strip_const_memsets(nc)
    nc.compile()
    return nc


def _get_nc():
    if "nc" not in _CACHE:
        _CACHE["nc"] = _build_nc()
    return _CACHE["nc"]


def _make_in_maps(A, B):
    Au = _unpack_upper(A)
    Bu = _unpack_upper(B)
    aT = np.ascontiguousarray(Au.T)  # aT[k, m] = Au[m, k]
    sdt = _store_np_dtype()
    aTk = aT.reshape(KT, P, N)  # [kt, p, m]
    Buk = Bu.reshape(KT, P, N)  # [kt, p, n]
    in_maps = []
    for units in ASSIGN:
        abarr = np.zeros((P, 2, UW), dtype=np.float32)
        for u, unit in enumerate(units):
            if unit is None:
                continue
            R, c, kt = unit
            abarr[:, u, :MB] = aTk[kt, :, R * MB : (R + 1) * MB]
            abarr[:, u, MB:] = Buk[kt, :, c * NB : (c + 1) * NB]
        in_maps.append({"ab": abarr.astype(sdt)})
    return in_maps


def _get_runner():
    """Build the sharded PJRT executable once; reuse across kernel() calls.

    Mirrors concourse.bass2jax.run_bass_via_pjrt's multi-core path, but
    caches the jitted function so repeat calls skip retracing.
    """
    if "runner" in _CACHE:
        return _CACHE["runner"]
    import jax
    import concourse.mybir as mybir
    from concourse import bass2jax
    from jax.experimental.shard_map import shard_map
    from jax.sharding import Mesh, PartitionSpec

    nc = _get_nc()
    bass2jax.install_neuronx_cc_hook()
    partition_name = (
        nc.partition_id_tensor.name if nc.partition_id_tensor else None
    )
    in_names, out_names, out_avals, zero_outs = [], [], [], []
    for alloc in nc.m.functions[0].allocations:
        if not isinstance(alloc, mybir.MemoryLocationSet):
            continue
        name = alloc.memorylocations[0].name
        if alloc.kind == "ExternalInput":
            if name != partition_name:
                in_names.append(name)
        elif alloc.kind == "ExternalOutput":
            out_names.append(name)
            shape = tuple(alloc.tensor_shape)
            dtype = mybir.dt.np(alloc.dtype)
            out_avals.append(jax.core.ShapedArray(shape, dtype))
            zero_outs.append(np.zeros(shape, dtype))
    n_params = len(in_names)
    n_outs = len(out_names)
    all_in = in_names + out_names + ([partition_name] if partition_name else [])
    donate = tuple(range(n_params, n_params + n_outs))

    def _body(*args):
        operands = list(args)
        if partition_name is not None:
            operands.append(bass2jax.partition_id_tensor())
        outs = bass2jax._bass_exec_p.bind(
            *operands,
            out_avals=tuple(out_avals),
            in_names=tuple(all_in),
            out_names=tuple(out_names),
            lowering_input_output_aliases=(),
            sim_require_finite=True,
            sim_require_nnan=True,
            nc=nc,
        )
        return tuple(outs)

    devices = jax.devices()[:NCORES]
    mesh = Mesh(np.asarray(devices), ("core",))
    fn = jax.jit(
        shard_map(
            _body,
            mesh=mesh,
            in_specs=(PartitionSpec("core"),) * (n_params + n_outs),
            out_specs=(PartitionSpec("core"),) * n_outs,
            check_rep=False,
        ),
        donate_argnums=donate,
        keep_unused=True,
    )
    runner = dict(
        fn=fn, in_names=in_names, out_names=out_names, zero_outs=zero_outs
    )
    _CACHE["runner"] = runner
    return runner


def _run_concat(concat_in):
    """Execute on 8 cores given axis-0-concatenated per-core inputs."""
    r = _get_runner()
    concat_zeros = [
        np.zeros((NCORES * z.shape[0], *z.shape[1:]), z.dtype)
        for z in r["zero_outs"]
    ]
    return r["fn"](*concat_in, *concat_zeros)


def _concat_inputs(in_maps):
    r = _get_runner()
    return [
        np.concatenate([in_maps[c][n] for c in range(NCORES)], axis=0)
        for n in r["in_names"]
    ]


def _assemble(out0):
    blocks = np.asarray(out0).reshape(NCORES, MB, NB)
    C = np.zeros((N, N), dtype=np.float32)
    for (R, c), cores in BLOCK_CORES.items():
        acc = blocks[cores[0]].copy()
        for g in cores[1:]:
            acc += blocks[g]
        C[R * MB : (R + 1) * MB, c * NB : (c + 1) * NB] = acc
    return C


def kernel(A, B):
    in_maps = _make_in_maps(A, B)
    concat_in = _concat_inputs(in_maps)
    out = _run_concat(concat_in)
    return _assemble(out[0])


# revision 21
# speedup vs baseline: 15636.3927x; 1.0116x over previous
"""Trainium2 Bass kernel: C = Au @ Bu for packed upper-triangular Au, Bu.

Inputs (full): A, B — packed row-major upper-triangular storage of two
512x512 f32 matrices, each a flat array of length 131328 = 512*513/2.
Output: dense [512, 512] f32 C = unpack(A) @ unpack(B)  (upper triangular).

Strategy — balanced triangular decomposition over a 4x2 C-block grid:
  C is tiled into 8 blocks of [128, 256].  Block (R, c) only needs
  contraction k in [128R, 256(c+1)) because A/B are upper triangular, so
  of the 32 (block, k-block) products only 13 are nonzero.  Those 13 MM
  units are spread over the 8 cores (<=2 each, padded with zero slabs),
  each unit = one 128-deep PE matmul [128k x 128m] @ [128k x 256n]
  accumulating in PSUM.  Blocks (2,0) and (3,0) are structurally zero;
  blocks (0,1) and (1,1) are split across two cores and summed on host.

  Per core: 384KB f32 in (vs 1.15MB for the dense column-shard), 2
  native fp32 matmuls (exact vs the f32 reference up to partial-sum
  ordering), one DVE PSUM->SBUF copy, one 128KB output DMA.  Raw bacc
  program (no Tile ceremony); entry const-AP memsets stripped since they
  gate the entry all-engine barrier on the Pool engine.
"""

import numpy as np

N = 512
P = 128
KT = 4  # k-blocks in the full problem
NCORES = 8
MB = 128  # C-block rows
NB = 256  # C-block cols
UW = MB + NB  # one unit slab: A part [128,128] + B part [128,256]
DTYPE = "f32"  # "bf16" | "f32" | "f32r"
PACKED_LEN = N * (N + 1) // 2

# core -> (unit0, unit1); unit = (R, c, kt) C-block row-band/col-band/k-block,
# or None for a zero-padded slot.
ASSIGN = [
    ((0, 0, 0), (0, 0, 1)),
    ((0, 1, 0), (0, 1, 1)),
    ((0, 1, 2), (0, 1, 3)),
    ((1, 1, 1), (1, 1, 2)),
    ((1, 1, 3), None),
    ((2, 1, 2), (2, 1, 3)),
    ((1, 0, 1), None),
    ((3, 1, 3), None),
]
# C block (R, c) -> list of cores whose outputs sum to it.
BLOCK_CORES = {}
for _g, _units in enumerate(ASSIGN):
    for _u in _units:
        if _u is not None:
            BLOCK_CORES.setdefault((_u[0], _u[1]), []).append(_g)
BLOCK_CORES = {k: sorted(set(v)) for k, v in BLOCK_CORES.items()}

_CACHE = {}


def _unpack_upper(p):
    """Packed row-major upper-tri -> dense [N, N] with zero lower triangle."""
    p = np.asarray(p, dtype=np.float32).reshape(-1)
    i = np.arange(N)[:, None]
    j = np.arange(N)[None, :]
    mask = j >= i
    pidx = np.where(mask, (i * (2 * N - i + 1)) // 2 + (j - i), 0)
    return np.where(mask, p[pidx], np.float32(0.0))


def _store_np_dtype():
    if DTYPE == "bf16":
        import ml_dtypes

        return ml_dtypes.bfloat16
    return np.float32


def _strip_const_memsets(nc):
    """Remove framework ceremony that isn't needed by this kernel:
    - the 4 unused const-AP memsets in the entry block (they gate the
      entry all-engine barrier on the Pool engine by ~400ns), and
    - the exit all-engine barrier EventSemaphores (the final SP
      wait_ge(osem) already guarantees the output landed; per-engine
      drains are kept)."""
    import concourse.mybir as mybir

    bb = nc.m.functions[0].blocks[0]
    bb.instructions = [
        i
        for i in bb.instructions
        if not (
            isinstance(i, mybir.InstMemset)
            and i.outs
            and "const-" in str(getattr(i.outs[0].bass_ap.tensor, "name", ""))
        )
    ]
    for bb in nc.m.functions[0].blocks:
        if bb.name.endswith("_end"):
            bb.instructions = [
                i
                for i in bb.instructions
                if not (
                    isinstance(i, mybir.InstEventSemaphore)
                    and str(i.name).startswith("aeb_barrier")
                )
            ]


def _build_nc():
    import concourse.mybir as mybir
    from concourse import bacc

    F32 = mybir.dt.float32
    store_dt = {
        "bf16": mybir.dt.bfloat16,
        "f32": F32,
        "f32r": mybir.dt.float32r,
    }[DTYPE]

    nc = bacc.Bacc("TRN2", num_devices=NCORES)
    ab = nc.dram_tensor("ab", [P, 2, UW], store_dt, kind="ExternalInput")
    cdr = nc.dram_tensor("c", [MB, NB], F32, kind="ExternalOutput")

    with (
        nc.sbuf_tensor([P, 2, UW], store_dt) as t,
        nc.sbuf_tensor([MB, NB], F32) as ostage,
        # Two PSUM tensors (one per output column half) so each half's
        # accumulation group closes independently — CoreSim tracks groups
        # per tensor, and the half-0 copy must not read into half-1's
        # still-open group.
        nc.psum_tensor([MB, NB // 2], F32) as psum_l,
        nc.psum_tensor([MB, NB // 2], F32) as psum_r,
        nc.semaphore("dsem0") as dsem0,
        nc.semaphore("dsem1") as dsem1,
        nc.semaphore("osem") as osem,
        nc.semaphore("psem") as psem,
        nc.semaphore("vsem") as vsem,
        nc.Block(no_gpsimd_drain=True) as block,
    ):
        # One semaphore per input chunk: two DMAs sharing one sem could
        # interleave per-engine completions, so a wait at 16 wouldn't prove
        # chunk 0 fully landed (CoreSim race detector flags this).
        dsems = [dsem0, dsem1]

        HN = NB // 2

        @block.sync
        def _(sync):
            for u in range(2):
                sync.dma_start(out=t.ap()[:, u], in_=ab.ap()[:, u]).then_inc(
                    dsems[u], 16
                )
            # Output split into column halves so half 0's copy + store
            # overlap half 1's matmuls/copy.  Both DMAs tick osem; the
            # final wait is for the total, so completion order is free.
            for h in range(2):
                sync.wait_ge(vsem, h + 1)
                sync.dma_start(
                    out=cdr.ap()[:, h * HN : (h + 1) * HN],
                    in_=ostage.ap()[:, h * HN : (h + 1) * HN],
                ).then_inc(osem, 16)
            sync.wait_ge(osem, 32)

        @block.tensor
        def _(tensor):
            # Column-half-split matmuls, ordered so the left half finishes
            # (and can be copied/stored) while the right half still runs.
            # PSUM accumulation groups are per column half (start at u0,
            # stop at u1); has_written state is per element, so the
            # interleaved groups in one bank are fine.
            psums = [psum_l, psum_r]
            seen = set()
            for u, h in [(0, 0), (1, 0), (0, 1), (1, 1)]:
                if u not in seen:
                    tensor.wait_ge(dsems[u], 16)
                    seen.add(u)
                mm = nc.tensor.matmul(
                    psums[h].ap(),
                    t.ap()[:, u, :MB],
                    t.ap()[:, u, MB + h * HN : MB + (h + 1) * HN],
                    start=(u == 0),
                    stop=(u == 1),
                )
                if u == 1:
                    mm.then_inc(psem, 1)

        @block.vector
        def _(vector):
            for h, ps in enumerate((psum_l, psum_r)):
                vector.wait_ge(psem, h + 1)
                nc.vector.tensor_copy(
                    ostage.ap()[:, h * HN : (h + 1) * HN], ps.ap()
                ).then_inc(vsem, 1)

    _strip_const_memsets(nc)
    nc.compile()
    return nc


def _get_nc():
    if "nc" not in _CACHE:
        _CACHE["nc"] = _build_nc()
    return _CACHE["nc"]


def _make_in_maps(A, B):
    Au = _unpack_upper(A)
    Bu = _unpack_upper(B)
    aT = np.ascontiguousarray(Au.T)  # aT[k, m] = Au[m, k]
    sdt = _store_np_dtype()
    aTk = aT.reshape(KT, P, N)  # [kt, p, m]
    Buk = Bu.reshape(KT, P, N)  # [kt, p, n]
    in_maps = []
    for units in ASSIGN:
        abarr = np.zeros((P, 2, UW), dtype=np.float32)
        for u, unit in enumerate(units):
            if unit is None:
                continue
            R, c, kt = unit
            abarr[:, u, :MB] = aTk[kt, :, R * MB : (R + 1) * MB]
            abarr[:, u, MB:] = Buk[kt, :, c * NB : (c + 1) * NB]
        in_maps.append({"ab": abarr.astype(sdt)})
    return in_maps


def _get_runner():
    """Build the sharded PJRT executable once; reuse across kernel() calls.

    Mirrors concourse.bass2jax.run_bass_via_pjrt's multi-core path, but
    caches the jitted function so repeat calls skip retracing.
    """
    if "runner" in _CACHE:
        return _CACHE["runner"]
    import jax
    import concourse.mybir as mybir
    from concourse import bass2jax
    from jax.experimental.shard_map import shard_map
    from jax.sharding import Mesh, PartitionSpec

    nc = _get_nc()
    bass2jax.install_neuronx_cc_hook()
    partition_name = (
        nc.partition_id_tensor.name if nc.partition_id_tensor else None
    )
    in_names, out_names, out_avals, zero_outs = [], [], [], []
    for alloc in nc.m.functions[0].allocations:
        if not isinstance(alloc, mybir.MemoryLocationSet):
            continue
        name = alloc.memorylocations[0].name
        if alloc.kind == "ExternalInput":
            if name != partition_name:
                in_names.append(name)
        elif alloc.kind == "ExternalOutput":
            out_names.append(name)
            shape = tuple(alloc.tensor_shape)
            dtype = mybir.dt.np(alloc.dtype)
            out_avals.append(jax.core.ShapedArray(shape, dtype))
            zero_outs.append(np.zeros(shape, dtype))
    n_params = len(in_names)
    n_outs = len(out_names)
    all_in = in_names + out_names + ([partition_name] if partition_name else [])
    donate = tuple(range(n_params, n_params + n_outs))

    def _body(*args):
        operands = list(args)
        if partition_name is not None:
            operands.append(bass2jax.partition_id_tensor())
        outs = bass2jax._bass_exec_p.bind(
            *operands,
            out_avals=tuple(out_avals),
            in_names=tuple(all_in),
            out_names=tuple(out_names),
            lowering_input_output_aliases=(),
            sim_require_finite=True,
            sim_require_nnan=True,
            nc=nc,
        )
        return tuple(outs)

    devices = jax.devices()[:NCORES]
    mesh = Mesh(np.asarray(devices), ("core",))
    fn = jax.jit(
        shard_map(
            _body,
            mesh=mesh,
            in_specs=(PartitionSpec("core"),) * (n_params + n_outs),
            out_specs=(PartitionSpec("core"),) * n_outs,
            check_rep=False,
        ),
        donate_argnums=donate,
        keep_unused=True,
    )
    runner = dict(
        fn=fn, in_names=in_names, out_names=out_names, zero_outs=zero_outs
    )
    _CACHE["runner"] = runner
    return runner


def _run_concat(concat_in):
    """Execute on 8 cores given axis-0-concatenated per-core inputs."""
    r = _get_runner()
    concat_zeros = [
        np.zeros((NCORES * z.shape[0], *z.shape[1:]), z.dtype)
        for z in r["zero_outs"]
    ]
    return r["fn"](*concat_in, *concat_zeros)


def _concat_inputs(in_maps):
    r = _get_runner()
    return [
        np.concatenate([in_maps[c][n] for c in range(NCORES)], axis=0)
        for n in r["in_names"]
    ]


def _assemble(out0):
    blocks = np.asarray(out0).reshape(NCORES, MB, NB)
    C = np.zeros((N, N), dtype=np.float32)
    for (R, c), cores in BLOCK_CORES.items():
        acc = blocks[cores[0]].copy()
        for g in cores[1:]:
            acc += blocks[g]
        C[R * MB : (R + 1) * MB, c * NB : (c + 1) * NB] = acc
    return C


def kernel(A, B):
    in_maps = _make_in_maps(A, B)
    concat_in = _concat_inputs(in_maps)
    out = _run_concat(concat_in)
    return _assemble(out[0])


# revision 22
# speedup vs baseline: 16006.9909x; 1.0237x over previous
"""Trainium2 Bass kernel: C = Au @ Bu for packed upper-triangular Au, Bu.

Inputs (full): A, B — packed row-major upper-triangular storage of two
512x512 f32 matrices, each a flat array of length 131328 = 512*513/2.
Output: dense [512, 512] f32 C = unpack(A) @ unpack(B)  (upper triangular).

Strategy — balanced triangular decomposition over a 4x2 C-block grid:
  C is tiled into 8 blocks of [128, 256].  Block (R, c) only needs
  contraction k in [128R, 256(c+1)) because A/B are upper triangular, so
  of the 32 (block, k-block) products only 13 are nonzero.  Those 13 MM
  units are spread over the 8 cores (<=2 each, padded with zero slabs),
  each unit = one 128-deep PE matmul [128k x 128m] @ [128k x 256n]
  accumulating in PSUM.  Blocks (2,0) and (3,0) are structurally zero;
  blocks (0,1) and (1,1) are split across two cores and summed on host.

  Per core: 384KB f32 in (vs 1.15MB for the dense column-shard), 2
  native fp32 matmuls (exact vs the f32 reference up to partial-sum
  ordering), one DVE PSUM->SBUF copy, one 128KB output DMA.  Raw bacc
  program (no Tile ceremony); entry const-AP memsets stripped since they
  gate the entry all-engine barrier on the Pool engine.
"""

import numpy as np

N = 512
P = 128
KT = 4  # k-blocks in the full problem
NCORES = 8
MB = 128  # C-block rows
NB = 256  # C-block cols
UW = MB + NB  # one unit slab: A part [128,128] + B part [128,256]
DTYPE = "f32"  # "bf16" | "f32" | "f32r"
PACKED_LEN = N * (N + 1) // 2

# core -> (unit0, unit1); unit = (R, c, kt) C-block row-band/col-band/k-block,
# or None for a zero-padded slot.
ASSIGN = [
    ((0, 0, 0), (0, 0, 1)),
    ((0, 1, 0), (0, 1, 1)),
    ((0, 1, 2), (0, 1, 3)),
    ((1, 1, 1), (1, 1, 2)),
    ((1, 1, 3), None),
    ((2, 1, 2), (2, 1, 3)),
    ((1, 0, 1), None),
    ((3, 1, 3), None),
]
# C block (R, c) -> list of cores whose outputs sum to it.
BLOCK_CORES = {}
for _g, _units in enumerate(ASSIGN):
    for _u in _units:
        if _u is not None:
            BLOCK_CORES.setdefault((_u[0], _u[1]), []).append(_g)
BLOCK_CORES = {k: sorted(set(v)) for k, v in BLOCK_CORES.items()}

_CACHE = {}


def _unpack_upper(p):
    """Packed row-major upper-tri -> dense [N, N] with zero lower triangle."""
    p = np.asarray(p, dtype=np.float32).reshape(-1)
    i = np.arange(N)[:, None]
    j = np.arange(N)[None, :]
    mask = j >= i
    pidx = np.where(mask, (i * (2 * N - i + 1)) // 2 + (j - i), 0)
    return np.where(mask, p[pidx], np.float32(0.0))


def _store_np_dtype():
    if DTYPE == "bf16":
        import ml_dtypes

        return ml_dtypes.bfloat16
    return np.float32


def _strip_const_memsets(nc):
    """Remove framework ceremony that isn't needed by this kernel:
    - the 4 unused const-AP memsets in the entry block (they gate the
      entry all-engine barrier on the Pool engine by ~400ns), and
    - the exit all-engine barrier EventSemaphores (the final SP
      wait_ge(osem) already guarantees the output landed; per-engine
      drains are kept)."""
    import concourse.mybir as mybir

    bb = nc.m.functions[0].blocks[0]
    bb.instructions = [
        i
        for i in bb.instructions
        if not (
            isinstance(i, mybir.InstMemset)
            and i.outs
            and "const-" in str(getattr(i.outs[0].bass_ap.tensor, "name", ""))
        )
    ]
    for bb in nc.m.functions[0].blocks:
        if bb.name.endswith("_end"):
            bb.instructions = [
                i
                for i in bb.instructions
                if not (
                    isinstance(i, mybir.InstEventSemaphore)
                    and str(i.name).startswith("aeb_barrier")
                )
            ]


def _build_nc():
    import concourse.mybir as mybir
    from concourse import bacc

    F32 = mybir.dt.float32
    store_dt = {
        "bf16": mybir.dt.bfloat16,
        "f32": F32,
        "f32r": mybir.dt.float32r,
    }[DTYPE]

    nc = bacc.Bacc("TRN2", num_devices=NCORES)
    ab = nc.dram_tensor("ab", [P, 2, UW], store_dt, kind="ExternalInput")
    cdr = nc.dram_tensor("c", [MB, NB], F32, kind="ExternalOutput")

    HN = NB // 2
    with (
        nc.sbuf_tensor([P, 2, UW], store_dt) as t,
        nc.sbuf_tensor([MB, NB], F32) as ostage,
        # Two PSUM tensors (one per output column half) so each half's
        # accumulation group closes independently — CoreSim tracks groups
        # per tensor, and the half-0 copy must not read into half-1's
        # still-open group.
        nc.psum_tensor([MB, HN], F32) as psum_l,
        nc.psum_tensor([MB, HN], F32) as psum_r,
        nc.semaphore("dsem0") as dsem0,
        nc.semaphore("dsem1") as dsem1,
        nc.semaphore("dsem2") as dsem2,
        nc.semaphore("osem") as osem,
        nc.semaphore("psem") as psem,
        nc.semaphore("vsem") as vsem,
        nc.Block(no_gpsimd_drain=True) as block,
    ):
        # Unit slab layout: [A (128) | B-left (128) | B-right (128)].
        # Input in 3 chunks sized to each matmul's minimal dependency:
        #   c0 = unit0 whole, c1 = unit1 A+B-left, c2 = unit1 B-right.
        # One semaphore per chunk: DMAs sharing one sem could interleave
        # per-engine completions, so a cumulative wait wouldn't prove an
        # individual chunk landed (CoreSim race detector flags this).
        dsems = [dsem0, dsem1, dsem2]
        chunks = [
            (t.ap()[:, 0], ab.ap()[:, 0]),
            (t.ap()[:, 1, : MB + HN], ab.ap()[:, 1, : MB + HN]),
            (t.ap()[:, 1, MB + HN :], ab.ap()[:, 1, MB + HN :]),
        ]

        @block.sync
        def _(sync):
            for d, (dst, src) in enumerate(chunks):
                sync.dma_start(out=dst, in_=src).then_inc(dsems[d], 16)
            # Output split into column halves so half 0's copy + store
            # overlap half 1's matmuls/copy.  Both DMAs tick osem; the
            # final wait is for the total, so completion order is free.
            for h in range(2):
                sync.wait_ge(vsem, h + 1)
                sync.dma_start(
                    out=cdr.ap()[:, h * HN : (h + 1) * HN],
                    in_=ostage.ap()[:, h * HN : (h + 1) * HN],
                ).then_inc(osem, 16)
            sync.wait_ge(osem, 32)

        @block.tensor
        def _(tensor):
            # Column-half-split matmuls, ordered so the left half finishes
            # (and can be copied/stored) while the right half still runs.
            # Each matmul waits only on the chunk that carries its data.
            # (unit, half, chunk, psum, start, stop, inc_psem)
            mms = [
                (0, 0, 0, psum_l, True, False, False),
                (1, 0, 1, psum_l, False, True, True),
                (0, 1, 0, psum_r, True, False, False),
                (1, 1, 2, psum_r, False, True, True),
            ]
            waited = set()
            for u, h, ck, ps, st, sp, inc in mms:
                if ck not in waited:
                    tensor.wait_ge(dsems[ck], 16)
                    waited.add(ck)
                mm = nc.tensor.matmul(
                    ps.ap(),
                    t.ap()[:, u, :MB],
                    t.ap()[:, u, MB + h * HN : MB + (h + 1) * HN],
                    start=st,
                    stop=sp,
                )
                if inc:
                    mm.then_inc(psem, 1)

        @block.vector
        def _(vector):
            for h, ps in enumerate((psum_l, psum_r)):
                vector.wait_ge(psem, h + 1)
                nc.vector.tensor_copy(
                    ostage.ap()[:, h * HN : (h + 1) * HN], ps.ap()
                ).then_inc(vsem, 1)

    _strip_const_memsets(nc)
    nc.compile()
    return nc


def _get_nc():
    if "nc" not in _CACHE:
        _CACHE["nc"] = _build_nc()
    return _CACHE["nc"]


def _make_in_maps(A, B):
    Au = _unpack_upper(A)
    Bu = _unpack_upper(B)
    aT = np.ascontiguousarray(Au.T)  # aT[k, m] = Au[m, k]
    sdt = _store_np_dtype()
    aTk = aT.reshape(KT, P, N)  # [kt, p, m]
    Buk = Bu.reshape(KT, P, N)  # [kt, p, n]
    in_maps = []
    for units in ASSIGN:
        abarr = np.zeros((P, 2, UW), dtype=np.float32)
        for u, unit in enumerate(units):
            if unit is None:
                continue
            R, c, kt = unit
            abarr[:, u, :MB] = aTk[kt, :, R * MB : (R + 1) * MB]
            abarr[:, u, MB:] = Buk[kt, :, c * NB : (c + 1) * NB]
        in_maps.append({"ab": abarr.astype(sdt)})
    return in_maps


def _get_runner():
    """Build the sharded PJRT executable once; reuse across kernel() calls.

    Mirrors concourse.bass2jax.run_bass_via_pjrt's multi-core path, but
    caches the jitted function so repeat calls skip retracing.
    """
    if "runner" in _CACHE:
        return _CACHE["runner"]
    import jax
    import concourse.mybir as mybir
    from concourse import bass2jax
    from jax.experimental.shard_map import shard_map
    from jax.sharding import Mesh, PartitionSpec

    nc = _get_nc()
    bass2jax.install_neuronx_cc_hook()
    partition_name = (
        nc.partition_id_tensor.name if nc.partition_id_tensor else None
    )
    in_names, out_names, out_avals, zero_outs = [], [], [], []
    for alloc in nc.m.functions[0].allocations:
        if not isinstance(alloc, mybir.MemoryLocationSet):
            continue
        name = alloc.memorylocations[0].name
        if alloc.kind == "ExternalInput":
            if name != partition_name:
                in_names.append(name)
        elif alloc.kind == "ExternalOutput":
            out_names.append(name)
            shape = tuple(alloc.tensor_shape)
            dtype = mybir.dt.np(alloc.dtype)
            out_avals.append(jax.core.ShapedArray(shape, dtype))
            zero_outs.append(np.zeros(shape, dtype))
    n_params = len(in_names)
    n_outs = len(out_names)
    all_in = in_names + out_names + ([partition_name] if partition_name else [])
    donate = tuple(range(n_params, n_params + n_outs))

    def _body(*args):
        operands = list(args)
        if partition_name is not None:
            operands.append(bass2jax.partition_id_tensor())
        outs = bass2jax._bass_exec_p.bind(
            *operands,
            out_avals=tuple(out_avals),
            in_names=tuple(all_in),
            out_names=tuple(out_names),
            lowering_input_output_aliases=(),
            sim_require_finite=True,
            sim_require_nnan=True,
            nc=nc,
        )
        return tuple(outs)

    devices = jax.devices()[:NCORES]
    mesh = Mesh(np.asarray(devices), ("core",))
    fn = jax.jit(
        shard_map(
            _body,
            mesh=mesh,
            in_specs=(PartitionSpec("core"),) * (n_params + n_outs),
            out_specs=(PartitionSpec("core"),) * n_outs,
            check_rep=False,
        ),
        donate_argnums=donate,
        keep_unused=True,
    )
    runner = dict(
        fn=fn, in_names=in_names, out_names=out_names, zero_outs=zero_outs
    )
    _CACHE["runner"] = runner
    return runner


def _run_concat(concat_in):
    """Execute on 8 cores given axis-0-concatenated per-core inputs."""
    r = _get_runner()
    concat_zeros = [
        np.zeros((NCORES * z.shape[0], *z.shape[1:]), z.dtype)
        for z in r["zero_outs"]
    ]
    return r["fn"](*concat_in, *concat_zeros)


def _concat_inputs(in_maps):
    r = _get_runner()
    return [
        np.concatenate([in_maps[c][n] for c in range(NCORES)], axis=0)
        for n in r["in_names"]
    ]


def _assemble(out0):
    blocks = np.asarray(out0).reshape(NCORES, MB, NB)
    C = np.zeros((N, N), dtype=np.float32)
    for (R, c), cores in BLOCK_CORES.items():
        acc = blocks[cores[0]].copy()
        for g in cores[1:]:
            acc += blocks[g]
        C[R * MB : (R + 1) * MB, c * NB : (c + 1) * NB] = acc
    return C


def kernel(A, B):
    in_maps = _make_in_maps(A, B)
    concat_in = _concat_inputs(in_maps)
    out = _run_concat(concat_in)
    return _assemble(out[0])


# revision 23
# speedup vs baseline: 16443.8372x; 1.0273x over previous
"""Trainium2 Bass kernel: C = Au @ Bu for packed upper-triangular Au, Bu.

Inputs (full): A, B — packed row-major upper-triangular storage of two
512x512 f32 matrices, each a flat array of length 131328 = 512*513/2.
Output: dense [512, 512] f32 C = unpack(A) @ unpack(B)  (upper triangular).

Strategy — balanced triangular brick decomposition:
  C is tiled into [128, 128] bricks.  Brick (R, nb) only needs
  contraction k-blocks kt in [R, nb] (A is upper-tri -> k >= 128R;
  B is upper-tri -> k <= 128nb+127), so of the 64 (brick, kt) products
  only 20 are nonzero.  Those 20 MM bricks are spread over the 8 cores
  (3 slots each, zero-padded), every brick an independent
  [128k x 128m] @ [128k x 128n] native-fp32 PE matmul.  Bricks of the
  same (R, nb) land on PSUM/host as partial sums and are added during
  unsharding (host add; error ~1 ulp vs a single fp32 accumulation).

  Per core: 3x 128KB input chunks (one per brick, own semaphore so each
  matmul starts at its minimal dependency), 3 matmuls into 3 PSUM
  tensors, per-brick DVE copy and per-brick output DMA so the store
  pipeline drains while later bricks still compute.  Raw bacc program
  (no Tile ceremony); the entry const-AP memsets and exit all-engine
  barrier (unneeded here) are stripped from the IR.
"""

import numpy as np

N = 512
P = 128
KT = 4  # k-blocks in the full problem
NCORES = 8
S = 3  # brick slots per core
BW = 256  # slab cols per slot: A part 128 + B part 128
PACKED_LEN = N * (N + 1) // 2

# core -> (slot0, slot1, slot2); slot = (R, nb, kt) brick coordinates
# (C rows 128R.., cols 128nb.., contraction k-block kt), or None for a
# zero-padded slot.
ASSIGN = [
    ((0, 3, 0), (0, 3, 1), (0, 3, 2)),
    ((0, 3, 3), (1, 3, 1), (1, 3, 2)),
    ((1, 3, 3), (2, 3, 2), (2, 3, 3)),
    ((3, 3, 3), (0, 2, 0), (0, 2, 1)),
    ((0, 2, 2), (1, 2, 1), (1, 2, 2)),
    ((2, 2, 2), (0, 1, 0), (0, 1, 1)),
    ((1, 1, 1), (0, 0, 0), None),
    (None, None, None),
]
# C brick (R, nb) -> list of (core, slot) contributions to sum.
BRICK_SRC = {}
for _g, _slots in enumerate(ASSIGN):
    for _s, _u in enumerate(_slots):
        if _u is not None:
            BRICK_SRC.setdefault((_u[0], _u[1]), []).append((_g, _s))

_CACHE = {}


def _unpack_upper(p):
    """Packed row-major upper-tri -> dense [N, N] with zero lower triangle."""
    p = np.asarray(p, dtype=np.float32).reshape(-1)
    i = np.arange(N)[:, None]
    j = np.arange(N)[None, :]
    mask = j >= i
    pidx = np.where(mask, (i * (2 * N - i + 1)) // 2 + (j - i), 0)
    return np.where(mask, p[pidx], np.float32(0.0))


def _strip_framework_ceremony(nc):
    """Remove framework ceremony that isn't needed by this kernel:
    - the 4 unused const-AP memsets in the entry block (they gate the
      entry all-engine barrier on the Pool engine by ~400ns), and
    - the exit all-engine barrier EventSemaphores (the final SP
      wait_ge(osem) already guarantees the output landed; per-engine
      drains are kept).  Verified repeat-execution safe: the runtime
      resets semaphores between executions."""
    import concourse.mybir as mybir

    bb = nc.m.functions[0].blocks[0]
    bb.instructions = [
        i
        for i in bb.instructions
        if not (
            isinstance(i, mybir.InstMemset)
            and i.outs
            and "const-" in str(getattr(i.outs[0].bass_ap.tensor, "name", ""))
        )
    ]
    for bb in nc.m.functions[0].blocks:
        if bb.name.endswith("_end"):
            bb.instructions = [
                i
                for i in bb.instructions
                if not (
                    isinstance(i, mybir.InstEventSemaphore)
                    and str(i.name).startswith("aeb_barrier")
                )
            ]


def _build_nc():
    import concourse.mybir as mybir
    from concourse import bacc

    F32 = mybir.dt.float32

    nc = bacc.Bacc("TRN2", num_devices=NCORES)
    ab = nc.dram_tensor("ab", [P, S, BW], F32, kind="ExternalInput")
    cdr = nc.dram_tensor("c", [P, S, 128], F32, kind="ExternalOutput")

    with (
        nc.sbuf_tensor([P, S, BW], F32) as t,
        nc.sbuf_tensor([P, S, 128], F32) as ostage,
        # One PSUM tensor per slot: independent accumulation groups, and
        # each slot's copy must not read another slot's open group.
        nc.psum_tensor([P, 128], F32) as ps0,
        nc.psum_tensor([P, 128], F32) as ps1,
        nc.psum_tensor([P, 128], F32) as ps2,
        # One semaphore per input chunk: DMAs sharing one sem could
        # interleave per-engine completions, so a cumulative wait wouldn't
        # prove an individual chunk landed (CoreSim race detector).
        nc.semaphore("ds0") as ds0,
        nc.semaphore("ds1") as ds1,
        nc.semaphore("ds2") as ds2,
        nc.semaphore("osem") as osem,
        nc.semaphore("psem") as psem,
        nc.semaphore("vsem") as vsem,
        nc.Block(no_gpsimd_drain=True) as block,
    ):
        dsems = [ds0, ds1, ds2]
        psums = [ps0, ps1, ps2]

        @block.sync
        def _(sync):
            for s in range(S):
                sync.dma_start(out=t.ap()[:, s], in_=ab.ap()[:, s]).then_inc(
                    dsems[s], 16
                )
            # Per-brick stores: slot s's output DMA launches as soon as its
            # copy lands, overlapping later bricks' matmuls/copies.  All
            # tick osem; the final wait is for the total.
            for s in range(S):
                sync.wait_ge(vsem, s + 1)
                sync.dma_start(
                    out=cdr.ap()[:, s], in_=ostage.ap()[:, s]
                ).then_inc(osem, 16)
            sync.wait_ge(osem, 16 * S)

        @block.tensor
        def _(tensor):
            for s in range(S):
                tensor.wait_ge(dsems[s], 16)
                nc.tensor.matmul(
                    psums[s].ap(),
                    t.ap()[:, s, :128],
                    t.ap()[:, s, 128:],
                    start=True,
                    stop=True,
                ).then_inc(psem, 1)

        @block.vector
        def _(vector):
            for s in range(S):
                vector.wait_ge(psem, s + 1)
                nc.vector.tensor_copy(
                    ostage.ap()[:, s], psums[s].ap()
                ).then_inc(vsem, 1)

    _strip_framework_ceremony(nc)
    nc.compile()
    return nc


def _get_nc():
    if "nc" not in _CACHE:
        _CACHE["nc"] = _build_nc()
    return _CACHE["nc"]


def _make_in_maps(A, B):
    Au = _unpack_upper(A)
    Bu = _unpack_upper(B)
    aT = np.ascontiguousarray(Au.T)  # aT[k, m] = Au[m, k]
    aTk = aT.reshape(KT, P, N)  # [kt, p, m]
    Buk = Bu.reshape(KT, P, N)  # [kt, p, n]
    in_maps = []
    for slots in ASSIGN:
        abarr = np.zeros((P, S, BW), dtype=np.float32)
        for s, unit in enumerate(slots):
            if unit is None:
                continue
            R, nb, kt = unit
            abarr[:, s, :128] = aTk[kt, :, R * P : (R + 1) * P]
            abarr[:, s, 128:] = Buk[kt, :, nb * P : (nb + 1) * P]
        in_maps.append({"ab": abarr})
    return in_maps


def _get_runner():
    """Build the sharded PJRT executable once; reuse across kernel() calls.

    Mirrors concourse.bass2jax.run_bass_via_pjrt's multi-core path, but
    caches the jitted function so repeat calls skip retracing.
    """
    if "runner" in _CACHE:
        return _CACHE["runner"]
    import jax
    import concourse.mybir as mybir
    from concourse import bass2jax
    from jax.experimental.shard_map import shard_map
    from jax.sharding import Mesh, PartitionSpec

    nc = _get_nc()
    bass2jax.install_neuronx_cc_hook()
    partition_name = (
        nc.partition_id_tensor.name if nc.partition_id_tensor else None
    )
    in_names, out_names, out_avals, zero_outs = [], [], [], []
    for alloc in nc.m.functions[0].allocations:
        if not isinstance(alloc, mybir.MemoryLocationSet):
            continue
        name = alloc.memorylocations[0].name
        if alloc.kind == "ExternalInput":
            if name != partition_name:
                in_names.append(name)
        elif alloc.kind == "ExternalOutput":
            out_names.append(name)
            shape = tuple(alloc.tensor_shape)
            dtype = mybir.dt.np(alloc.dtype)
            out_avals.append(jax.core.ShapedArray(shape, dtype))
            zero_outs.append(np.zeros(shape, dtype))
    n_params = len(in_names)
    n_outs = len(out_names)
    all_in = in_names + out_names + ([partition_name] if partition_name else [])
    donate = tuple(range(n_params, n_params + n_outs))

    def _body(*args):
        operands = list(args)
        if partition_name is not None:
            operands.append(bass2jax.partition_id_tensor())
        outs = bass2jax._bass_exec_p.bind(
            *operands,
            out_avals=tuple(out_avals),
            in_names=tuple(all_in),
            out_names=tuple(out_names),
            lowering_input_output_aliases=(),
            sim_require_finite=True,
            sim_require_nnan=True,
            nc=nc,
        )
        return tuple(outs)

    devices = jax.devices()[:NCORES]
    mesh = Mesh(np.asarray(devices), ("core",))
    fn = jax.jit(
        shard_map(
            _body,
            mesh=mesh,
            in_specs=(PartitionSpec("core"),) * (n_params + n_outs),
            out_specs=(PartitionSpec("core"),) * n_outs,
            check_rep=False,
        ),
        donate_argnums=donate,
        keep_unused=True,
    )
    runner = dict(
        fn=fn, in_names=in_names, out_names=out_names, zero_outs=zero_outs
    )
    _CACHE["runner"] = runner
    return runner


def _run_concat(concat_in):
    """Execute on 8 cores given axis-0-concatenated per-core inputs."""
    r = _get_runner()
    concat_zeros = [
        np.zeros((NCORES * z.shape[0], *z.shape[1:]), z.dtype)
        for z in r["zero_outs"]
    ]
    return r["fn"](*concat_in, *concat_zeros)


def _concat_inputs(in_maps):
    r = _get_runner()
    return [
        np.concatenate([in_maps[c][n] for c in range(NCORES)], axis=0)
        for n in r["in_names"]
    ]


def _assemble(out0):
    # out0: concat over cores of [P, S, 128] -> [NCORES, P(m), S, 128(n)]
    bricks = np.asarray(out0).reshape(NCORES, P, S, 128)
    C = np.zeros((N, N), dtype=np.float32)
    for (R, nb), srcs in BRICK_SRC.items():
        (g0, s0) = srcs[0]
        acc = bricks[g0, :, s0, :].copy()
        for g, s in srcs[1:]:
            acc += bricks[g, :, s, :]
        C[R * P : (R + 1) * P, nb * P : (nb + 1) * P] = acc
    return C


def kernel(A, B):
    in_maps = _make_in_maps(A, B)
    concat_in = _concat_inputs(in_maps)
    out = _run_concat(concat_in)
    return _assemble(out[0])


# revision 24
# speedup vs baseline: 17128.9971x; 1.0417x over previous
"""Trainium2 Bass kernel: C = Au @ Bu for packed upper-triangular Au, Bu.

Inputs (full): A, B — packed row-major upper-triangular storage of two
512x512 f32 matrices, each a flat array of length 131328 = 512*513/2.
Output: dense [512, 512] f32 C = unpack(A) @ unpack(B)  (upper triangular).

Strategy — balanced triangular brick decomposition:
  C is tiled into [128, 128] bricks.  Brick (R, nb) only needs
  contraction k-blocks kt in [R, nb] (A is upper-tri -> k >= 128R;
  B is upper-tri -> k <= 128nb+127), so of the 64 (brick, kt) products
  only 20 are nonzero.  Those 20 MM bricks are spread over the 8 cores
  (3 slots each, zero-padded), every brick an independent
  [128k x 128m] @ [128k x 128n] native-fp32 PE matmul.  Bricks of the
  same (R, nb) land on PSUM/host as partial sums and are added during
  unsharding (host add; error ~1 ulp vs a single fp32 accumulation).

  Per core: 3x 128KB input chunks (one per brick, own semaphore so each
  matmul starts at its minimal dependency), 3 matmuls into 3 PSUM
  tensors, per-brick DVE copy and per-brick output DMA so the store
  pipeline drains while later bricks still compute.  Raw bacc program
  (no Tile ceremony); the entry const-AP memsets and exit all-engine
  barrier (unneeded here) are stripped from the IR.
"""

import numpy as np

N = 512
P = 128
KT = 4  # k-blocks in the full problem
NCORES = 8
S = 3  # brick slots per core
BW = 256  # slab cols per slot: A part 128 + B part 128
PACKED_LEN = N * (N + 1) // 2

# core -> (slot0, slot1, slot2); slot = (R, nb, kt) brick coordinates
# (C rows 128R.., cols 128nb.., contraction k-block kt), or None for a
# zero-padded slot.
ASSIGN = [
    ((0, 3, 0), (0, 3, 1), (0, 3, 2)),
    ((0, 3, 3), (1, 3, 1), (1, 3, 2)),
    ((1, 3, 3), (2, 3, 2), (2, 3, 3)),
    ((3, 3, 3), (0, 2, 0), (0, 2, 1)),
    ((0, 2, 2), (1, 2, 1), (1, 2, 2)),
    ((2, 2, 2), (0, 1, 0), (0, 1, 1)),
    ((1, 1, 1), (0, 0, 0), None),
    (None, None, None),
]
# C brick (R, nb) -> list of (core, slot) contributions to sum.
BRICK_SRC = {}
for _g, _slots in enumerate(ASSIGN):
    for _s, _u in enumerate(_slots):
        if _u is not None:
            BRICK_SRC.setdefault((_u[0], _u[1]), []).append((_g, _s))

_CACHE = {}


def _unpack_upper(p):
    """Packed row-major upper-tri -> dense [N, N] with zero lower triangle."""
    p = np.asarray(p, dtype=np.float32).reshape(-1)
    i = np.arange(N)[:, None]
    j = np.arange(N)[None, :]
    mask = j >= i
    pidx = np.where(mask, (i * (2 * N - i + 1)) // 2 + (j - i), 0)
    return np.where(mask, p[pidx], np.float32(0.0))


def _strip_framework_ceremony(nc):
    """IR surgery on the built program:
    - drop the 4 unused const-AP memsets in the entry block (they gate
      the entry all-engine barrier on the Pool engine by ~400ns);
    - drop the exit all-engine barrier EventSemaphores (the final SP
      wait_ge(osem) already guarantees the output landed; per-engine
      drains are kept);
    - hoist the three input DMACopies to the head of the entry block,
      ahead of SP's entry-barrier participation, so descriptor
      generation and the transfers overlap the barrier (~300ns).  Safe:
      nothing reads the SBUF tile before its per-chunk semaphore fires,
      and the runtime resets semaphores between executions (verified by
      repeat runs)."""
    import concourse.mybir as mybir

    f = nc.m.functions[0]
    entry = f.blocks[0]
    entry.instructions = [
        i
        for i in entry.instructions
        if not (
            isinstance(i, mybir.InstMemset)
            and i.outs
            and "const-" in str(getattr(i.outs[0].bass_ap.tensor, "name", ""))
        )
    ]
    for bb in f.blocks:
        if bb.name.endswith("_end"):
            bb.instructions = [
                i
                for i in bb.instructions
                if not (
                    isinstance(i, mybir.InstEventSemaphore)
                    and str(i.name).startswith("aeb_barrier")
                )
            ]
    for bb in f.blocks:
        dmas = [
            i
            for i in bb.instructions
            if isinstance(i, mybir.InstDMACopy)
            and i.engine == mybir.EngineType.SP
            and i.outs
            and "t_" in str(getattr(i.outs[0].bass_ap.tensor, "name", ""))
        ]
        if dmas:
            ins = dmas[:S]
            bb.instructions = [i for i in bb.instructions if i not in ins]
            entry.instructions = ins + entry.instructions
            break


def _build_nc():
    import concourse.mybir as mybir
    from concourse import bacc

    F32 = mybir.dt.float32

    nc = bacc.Bacc("TRN2", num_devices=NCORES)
    ab = nc.dram_tensor("ab", [P, S, BW], F32, kind="ExternalInput")
    cdr = nc.dram_tensor("c", [P, S, 128], F32, kind="ExternalOutput")

    with (
        nc.sbuf_tensor([P, S, BW], F32) as t,
        nc.sbuf_tensor([P, S, 128], F32) as ostage,
        # One PSUM tensor per slot: independent accumulation groups, and
        # each slot's copy must not read another slot's open group.
        nc.psum_tensor([P, 128], F32) as ps0,
        nc.psum_tensor([P, 128], F32) as ps1,
        nc.psum_tensor([P, 128], F32) as ps2,
        # One semaphore per input chunk: DMAs sharing one sem could
        # interleave per-engine completions, so a cumulative wait wouldn't
        # prove an individual chunk landed (CoreSim race detector).
        nc.semaphore("ds0") as ds0,
        nc.semaphore("ds1") as ds1,
        nc.semaphore("ds2") as ds2,
        nc.semaphore("osem") as osem,
        nc.semaphore("psem") as psem,
        nc.semaphore("vsem") as vsem,
        nc.Block(no_gpsimd_drain=True) as block,
    ):
        dsems = [ds0, ds1, ds2]
        psums = [ps0, ps1, ps2]

        @block.sync
        def _(sync):
            for s in range(S):
                sync.dma_start(out=t.ap()[:, s], in_=ab.ap()[:, s]).then_inc(
                    dsems[s], 16
                )
            # Per-brick stores: slot s's output DMA launches as soon as its
            # copy lands, overlapping later bricks' matmuls/copies.  All
            # tick osem; the final wait is for the total.
            for s in range(S):
                sync.wait_ge(vsem, s + 1)
                sync.dma_start(
                    out=cdr.ap()[:, s], in_=ostage.ap()[:, s]
                ).then_inc(osem, 16)
            sync.wait_ge(osem, 16 * S)

        @block.tensor
        def _(tensor):
            for s in range(S):
                tensor.wait_ge(dsems[s], 16)
                nc.tensor.matmul(
                    psums[s].ap(),
                    t.ap()[:, s, :128],
                    t.ap()[:, s, 128:],
                    start=True,
                    stop=True,
                ).then_inc(psem, 1)

        @block.vector
        def _(vector):
            for s in range(S):
                vector.wait_ge(psem, s + 1)
                nc.vector.tensor_copy(
                    ostage.ap()[:, s], psums[s].ap()
                ).then_inc(vsem, 1)

    _strip_framework_ceremony(nc)
    nc.compile()
    return nc


def _get_nc():
    if "nc" not in _CACHE:
        _CACHE["nc"] = _build_nc()
    return _CACHE["nc"]


def _make_in_maps(A, B):
    Au = _unpack_upper(A)
    Bu = _unpack_upper(B)
    aT = np.ascontiguousarray(Au.T)  # aT[k, m] = Au[m, k]
    aTk = aT.reshape(KT, P, N)  # [kt, p, m]
    Buk = Bu.reshape(KT, P, N)  # [kt, p, n]
    in_maps = []
    for slots in ASSIGN:
        abarr = np.zeros((P, S, BW), dtype=np.float32)
        for s, unit in enumerate(slots):
            if unit is None:
                continue
            R, nb, kt = unit
            abarr[:, s, :128] = aTk[kt, :, R * P : (R + 1) * P]
            abarr[:, s, 128:] = Buk[kt, :, nb * P : (nb + 1) * P]
        in_maps.append({"ab": abarr})
    return in_maps


def _get_runner():
    """Build the sharded PJRT executable once; reuse across kernel() calls.

    Mirrors concourse.bass2jax.run_bass_via_pjrt's multi-core path, but
    caches the jitted function so repeat calls skip retracing.
    """
    if "runner" in _CACHE:
        return _CACHE["runner"]
    import jax
    import concourse.mybir as mybir
    from concourse import bass2jax
    from jax.experimental.shard_map import shard_map
    from jax.sharding import Mesh, PartitionSpec

    nc = _get_nc()
    bass2jax.install_neuronx_cc_hook()
    partition_name = (
        nc.partition_id_tensor.name if nc.partition_id_tensor else None
    )
    in_names, out_names, out_avals, zero_outs = [], [], [], []
    for alloc in nc.m.functions[0].allocations:
        if not isinstance(alloc, mybir.MemoryLocationSet):
            continue
        name = alloc.memorylocations[0].name
        if alloc.kind == "ExternalInput":
            if name != partition_name:
                in_names.append(name)
        elif alloc.kind == "ExternalOutput":
            out_names.append(name)
            shape = tuple(alloc.tensor_shape)
            dtype = mybir.dt.np(alloc.dtype)
            out_avals.append(jax.core.ShapedArray(shape, dtype))
            zero_outs.append(np.zeros(shape, dtype))
    n_params = len(in_names)
    n_outs = len(out_names)
    all_in = in_names + out_names + ([partition_name] if partition_name else [])
    donate = tuple(range(n_params, n_params + n_outs))

    def _body(*args):
        operands = list(args)
        if partition_name is not None:
            operands.append(bass2jax.partition_id_tensor())
        outs = bass2jax._bass_exec_p.bind(
            *operands,
            out_avals=tuple(out_avals),
            in_names=tuple(all_in),
            out_names=tuple(out_names),
            lowering_input_output_aliases=(),
            sim_require_finite=True,
            sim_require_nnan=True,
            nc=nc,
        )
        return tuple(outs)

    devices = jax.devices()[:NCORES]
    mesh = Mesh(np.asarray(devices), ("core",))
    fn = jax.jit(
        shard_map(
            _body,
            mesh=mesh,
            in_specs=(PartitionSpec("core"),) * (n_params + n_outs),
            out_specs=(PartitionSpec("core"),) * n_outs,
            check_rep=False,
        ),
        donate_argnums=donate,
        keep_unused=True,
    )
    runner = dict(
        fn=fn, in_names=in_names, out_names=out_names, zero_outs=zero_outs
    )
    _CACHE["runner"] = runner
    return runner


def _run_concat(concat_in):
    """Execute on 8 cores given axis-0-concatenated per-core inputs."""
    r = _get_runner()
    concat_zeros = [
        np.zeros((NCORES * z.shape[0], *z.shape[1:]), z.dtype)
        for z in r["zero_outs"]
    ]
    return r["fn"](*concat_in, *concat_zeros)


def _concat_inputs(in_maps):
    r = _get_runner()
    return [
        np.concatenate([in_maps[c][n] for c in range(NCORES)], axis=0)
        for n in r["in_names"]
    ]


def _assemble(out0):
    # out0: concat over cores of [P, S, 128] -> [NCORES, P(m), S, 128(n)]
    bricks = np.asarray(out0).reshape(NCORES, P, S, 128)
    C = np.zeros((N, N), dtype=np.float32)
    for (R, nb), srcs in BRICK_SRC.items():
        (g0, s0) = srcs[0]
        acc = bricks[g0, :, s0, :].copy()
        for g, s in srcs[1:]:
            acc += bricks[g, :, s, :]
        C[R * P : (R + 1) * P, nb * P : (nb + 1) * P] = acc
    return C


def kernel(A, B):
    in_maps = _make_in_maps(A, B)
    concat_in = _concat_inputs(in_maps)
    out = _run_concat(concat_in)
    return _assemble(out[0])


# revision 26
# speedup vs baseline: 17599.8973x; 1.0275x over previous
"""Trainium2 Bass kernel: C = Au @ Bu for packed upper-triangular Au, Bu.

Inputs (full): A, B — packed row-major upper-triangular storage of two
512x512 f32 matrices, each a flat array of length 131328 = 512*513/2.
Output: dense [512, 512] f32 C = unpack(A) @ unpack(B)  (upper triangular).

Strategy — balanced triangular brick decomposition:
  C is tiled into [128, 128] bricks.  Brick (R, nb) only needs
  contraction k-blocks kt in [R, nb] (A is upper-tri -> k >= 128R;
  B is upper-tri -> k <= 128nb+127), so of the 64 (brick, kt) products
  only 20 are nonzero.  Those 20 MM bricks are spread over the 8 cores
  (3 slots each, zero-padded), every brick an independent
  [128k x 128m] @ [128k x 128n] native-fp32 PE matmul.  Bricks of the
  same (R, nb) land on PSUM/host as partial sums and are added during
  unsharding (host add; error ~1 ulp vs a single fp32 accumulation).

  Per core: 3x 128KB input chunks (one per brick, own semaphore so each
  matmul starts at its minimal dependency), 3 matmuls into 3 PSUM
  tensors, per-brick DVE copy and per-brick output DMA so the store
  pipeline drains while later bricks still compute.  Raw bacc program
  (no Tile ceremony); the entry const-AP memsets and exit all-engine
  barrier (unneeded here) are stripped from the IR.
"""

import numpy as np

N = 512
P = 128
KT = 4  # k-blocks in the full problem
NCORES = 8
S = 3  # brick slots per core
BW = 256  # slab cols per slot: A part 128 + B part 128
PACKED_LEN = N * (N + 1) // 2

# core -> (slot0, slot1, slot2); slot = (R, nb, kt) brick coordinates
# (C rows 128R.., cols 128nb.., contraction k-block kt), or None for a
# zero-padded slot.
ASSIGN = [
    ((0, 3, 0), (0, 3, 1), (0, 3, 2)),
    ((0, 3, 3), (1, 3, 1), (1, 3, 2)),
    ((1, 3, 3), (2, 3, 2), (2, 3, 3)),
    ((3, 3, 3), (0, 2, 0), (0, 2, 1)),
    ((0, 2, 2), (1, 2, 1), (1, 2, 2)),
    ((2, 2, 2), (0, 1, 0), (0, 1, 1)),
    ((1, 1, 1), (0, 0, 0), None),
    (None, None, None),
]
# C brick (R, nb) -> list of (core, slot) contributions to sum.
BRICK_SRC = {}
for _g, _slots in enumerate(ASSIGN):
    for _s, _u in enumerate(_slots):
        if _u is not None:
            BRICK_SRC.setdefault((_u[0], _u[1]), []).append((_g, _s))

_CACHE = {}


def _unpack_upper(p):
    """Packed row-major upper-tri -> dense [N, N] with zero lower triangle."""
    p = np.asarray(p, dtype=np.float32).reshape(-1)
    i = np.arange(N)[:, None]
    j = np.arange(N)[None, :]
    mask = j >= i
    pidx = np.where(mask, (i * (2 * N - i + 1)) // 2 + (j - i), 0)
    return np.where(mask, p[pidx], np.float32(0.0))


def _strip_framework_ceremony(nc):
    """IR surgery on the built program:
    - drop the 4 unused const-AP memsets in the entry block (they gate
      the entry all-engine barrier on the Pool engine by ~400ns);
    - drop the exit all-engine barrier EventSemaphores (the final SP
      wait_ge(osem) already guarantees the output landed; per-engine
      drains are kept);
    - hoist the three input DMACopies to the head of the entry block,
      ahead of SP's entry-barrier participation, so descriptor
      generation and the transfers overlap the barrier (~300ns).  Safe:
      nothing reads the SBUF tile before its per-chunk semaphore fires,
      and the runtime resets semaphores between executions (verified by
      repeat runs)."""
    import concourse.mybir as mybir

    f = nc.m.functions[0]
    entry = f.blocks[0]
    entry.instructions = [
        i
        for i in entry.instructions
        if not (
            isinstance(i, mybir.InstMemset)
            and i.outs
            and "const-" in str(getattr(i.outs[0].bass_ap.tensor, "name", ""))
        )
    ]
    for bb in f.blocks:
        if bb.name.endswith("_end"):
            bb.instructions = [
                i
                for i in bb.instructions
                if not (
                    isinstance(i, mybir.InstEventSemaphore)
                    and str(i.name).startswith("aeb_barrier")
                )
            ]
    moved = []
    for bb in f.blocks:
        dmas = [
            i
            for i in bb.instructions
            if isinstance(i, mybir.InstDMACopy)
            and i.outs
            and "t_" in str(getattr(i.outs[0].bass_ap.tensor, "name", ""))
        ]
        if dmas:
            bb.instructions = [i for i in bb.instructions if i not in dmas]
            moved += dmas
    entry.instructions = moved + entry.instructions


def _build_nc():
    import concourse.mybir as mybir
    from concourse import bacc

    F32 = mybir.dt.float32

    nc = bacc.Bacc("TRN2", num_devices=NCORES)
    ab = nc.dram_tensor("ab", [P, S, BW], F32, kind="ExternalInput")
    cdr = nc.dram_tensor("c", [P, S, 128], F32, kind="ExternalOutput")

    with (
        nc.sbuf_tensor([P, S, BW], F32) as t,
        nc.sbuf_tensor([P, S, 128], F32) as ostage,
        # One PSUM tensor per slot: independent accumulation groups, and
        # each slot's copy must not read another slot's open group.
        nc.psum_tensor([P, 128], F32) as ps0,
        nc.psum_tensor([P, 128], F32) as ps1,
        nc.psum_tensor([P, 128], F32) as ps2,
        # One semaphore per input chunk: DMAs sharing one sem could
        # interleave per-engine completions, so a cumulative wait wouldn't
        # prove an individual chunk landed (CoreSim race detector).
        nc.semaphore("ds0") as ds0,
        nc.semaphore("ds1") as ds1,
        nc.semaphore("ds2") as ds2,
        nc.semaphore("osem") as osem,
        nc.semaphore("osem_p") as osem_p,
        nc.semaphore("psem") as psem,
        nc.semaphore("vsem") as vsem,
        nc.Block(no_gpsimd_drain=True) as block,
    ):
        dsems = [ds0, ds1, ds2]
        psums = [ps0, ps1, ps2]

        # Slot 1's input chunk and output store ride the SWDGE (Pool)
        # path: Q7 descriptor generation runs in parallel with the HWDGE
        # chain, so chunk 1's transfer slots between chunks 0/2 (earlier
        # receipts for MM1/MM2) and out 1 stays off the HWDGE chain that
        # gates out 2.  SWDGE requires its semaphore to start from 0, so
        # the Pool store ticks its own osem_p.
        @block.sync
        def _(sync):
            for s in (0, 2):
                sync.dma_start(out=t.ap()[:, s], in_=ab.ap()[:, s]).then_inc(
                    dsems[s], 16
                )
            # Per-brick stores: slot s's output DMA launches as soon as its
            # copy lands, overlapping later bricks' matmuls/copies.
            for s in (0, 2):
                sync.wait_ge(vsem, s + 1)
                sync.dma_start(
                    out=cdr.ap()[:, s], in_=ostage.ap()[:, s]
                ).then_inc(osem, 16)
            sync.wait_ge(osem, 32)
            sync.wait_ge(osem_p, 16)

        @block.gpsimd
        def _(gp):
            gp.dma_start(out=t.ap()[:, 1], in_=ab.ap()[:, 1]).then_inc(
                dsems[1], 16
            )
            gp.wait_ge(vsem, 2)
            gp.dma_start(out=cdr.ap()[:, 1], in_=ostage.ap()[:, 1]).then_inc(
                osem_p, 16
            )

        @block.tensor
        def _(tensor):
            for s in range(S):
                tensor.wait_ge(dsems[s], 16)
                nc.tensor.matmul(
                    psums[s].ap(),
                    t.ap()[:, s, :128],
                    t.ap()[:, s, 128:],
                    start=True,
                    stop=True,
                ).then_inc(psem, 1)

        @block.vector
        def _(vector):
            for s in range(S):
                vector.wait_ge(psem, s + 1)
                nc.vector.tensor_copy(
                    ostage.ap()[:, s], psums[s].ap()
                ).then_inc(vsem, 1)

    _strip_framework_ceremony(nc)
    nc.compile()
    return nc


def _get_nc():
    if "nc" not in _CACHE:
        _CACHE["nc"] = _build_nc()
    return _CACHE["nc"]


def _make_in_maps(A, B):
    Au = _unpack_upper(A)
    Bu = _unpack_upper(B)
    aT = np.ascontiguousarray(Au.T)  # aT[k, m] = Au[m, k]
    aTk = aT.reshape(KT, P, N)  # [kt, p, m]
    Buk = Bu.reshape(KT, P, N)  # [kt, p, n]
    in_maps = []
    for slots in ASSIGN:
        abarr = np.zeros((P, S, BW), dtype=np.float32)
        for s, unit in enumerate(slots):
            if unit is None:
                continue
            R, nb, kt = unit
            abarr[:, s, :128] = aTk[kt, :, R * P : (R + 1) * P]
            abarr[:, s, 128:] = Buk[kt, :, nb * P : (nb + 1) * P]
        in_maps.append({"ab": abarr})
    return in_maps


def _get_runner():
    """Build the sharded PJRT executable once; reuse across kernel() calls.

    Mirrors concourse.bass2jax.run_bass_via_pjrt's multi-core path, but
    caches the jitted function so repeat calls skip retracing.
    """
    if "runner" in _CACHE:
        return _CACHE["runner"]
    import jax
    import concourse.mybir as mybir
    from concourse import bass2jax
    from jax.experimental.shard_map import shard_map
    from jax.sharding import Mesh, PartitionSpec

    nc = _get_nc()
    bass2jax.install_neuronx_cc_hook()
    partition_name = (
        nc.partition_id_tensor.name if nc.partition_id_tensor else None
    )
    in_names, out_names, out_avals, zero_outs = [], [], [], []
    for alloc in nc.m.functions[0].allocations:
        if not isinstance(alloc, mybir.MemoryLocationSet):
            continue
        name = alloc.memorylocations[0].name
        if alloc.kind == "ExternalInput":
            if name != partition_name:
                in_names.append(name)
        elif alloc.kind == "ExternalOutput":
            out_names.append(name)
            shape = tuple(alloc.tensor_shape)
            dtype = mybir.dt.np(alloc.dtype)
            out_avals.append(jax.core.ShapedArray(shape, dtype))
            zero_outs.append(np.zeros(shape, dtype))
    n_params = len(in_names)
    n_outs = len(out_names)
    all_in = in_names + out_names + ([partition_name] if partition_name else [])
    donate = tuple(range(n_params, n_params + n_outs))

    def _body(*args):
        operands = list(args)
        if partition_name is not None:
            operands.append(bass2jax.partition_id_tensor())
        outs = bass2jax._bass_exec_p.bind(
            *operands,
            out_avals=tuple(out_avals),
            in_names=tuple(all_in),
            out_names=tuple(out_names),
            lowering_input_output_aliases=(),
            sim_require_finite=True,
            sim_require_nnan=True,
            nc=nc,
        )
        return tuple(outs)

    devices = jax.devices()[:NCORES]
    mesh = Mesh(np.asarray(devices), ("core",))
    fn = jax.jit(
        shard_map(
            _body,
            mesh=mesh,
            in_specs=(PartitionSpec("core"),) * (n_params + n_outs),
            out_specs=(PartitionSpec("core"),) * n_outs,
            check_rep=False,
        ),
        donate_argnums=donate,
        keep_unused=True,
    )
    runner = dict(
        fn=fn, in_names=in_names, out_names=out_names, zero_outs=zero_outs
    )
    _CACHE["runner"] = runner
    return runner


def _run_concat(concat_in):
    """Execute on 8 cores given axis-0-concatenated per-core inputs."""
    r = _get_runner()
    concat_zeros = [
        np.zeros((NCORES * z.shape[0], *z.shape[1:]), z.dtype)
        for z in r["zero_outs"]
    ]
    return r["fn"](*concat_in, *concat_zeros)


def _concat_inputs(in_maps):
    r = _get_runner()
    return [
        np.concatenate([in_maps[c][n] for c in range(NCORES)], axis=0)
        for n in r["in_names"]
    ]


def _assemble(out0):
    # out0: concat over cores of [P, S, 128] -> [NCORES, P(m), S, 128(n)]
    bricks = np.asarray(out0).reshape(NCORES, P, S, 128)
    C = np.zeros((N, N), dtype=np.float32)
    for (R, nb), srcs in BRICK_SRC.items():
        (g0, s0) = srcs[0]
        acc = bricks[g0, :, s0, :].copy()
        for g, s in srcs[1:]:
            acc += bricks[g, :, s, :]
        C[R * P : (R + 1) * P, nb * P : (nb + 1) * P] = acc
    return C


def kernel(A, B):
    in_maps = _make_in_maps(A, B)
    concat_in = _concat_inputs(in_maps)
    out = _run_concat(concat_in)
    return _assemble(out[0])


# revision 27
# speedup vs baseline: 17726.8075x; 1.0072x over previous
"""Trainium2 Bass kernel: C = Au @ Bu for packed upper-triangular Au, Bu.

Inputs (full): A, B — packed row-major upper-triangular storage of two
512x512 f32 matrices, each a flat array of length 131328 = 512*513/2.
Output: dense [512, 512] f32 C = unpack(A) @ unpack(B)  (upper triangular).

Strategy — balanced triangular brick decomposition:
  C is tiled into [128, 128] bricks.  Brick (R, nb) only needs
  contraction k-blocks kt in [R, nb] (A is upper-tri -> k >= 128R;
  B is upper-tri -> k <= 128nb+127), so of the 64 (brick, kt) products
  only 20 are nonzero.  Those 20 MM bricks are spread over the 8 cores
  (3 slots each, zero-padded), every brick an independent
  [128k x 128m] @ [128k x 128n] native-fp32 PE matmul.  Bricks of the
  same (R, nb) land on PSUM/host as partial sums and are added during
  unsharding (host add; error ~1 ulp vs a single fp32 accumulation).

  Per core: 3x 128KB input chunks (one per brick, own semaphore so each
  matmul starts at its minimal dependency), 3 matmuls into 3 PSUM
  tensors, per-brick DVE copy and per-brick output DMA so the store
  pipeline drains while later bricks still compute.  Raw bacc program
  (no Tile ceremony); the entry const-AP memsets and exit all-engine
  barrier (unneeded here) are stripped from the IR.
"""

import numpy as np

N = 512
P = 128
KT = 4  # k-blocks in the full problem
NCORES = 8
S = 3  # brick slots per core
BW = 256  # slab cols per slot: A part 128 + B part 128
PACKED_LEN = N * (N + 1) // 2

# core -> (slot0, slot1, slot2); slot = (R, nb, kt) brick coordinates
# (C rows 128R.., cols 128nb.., contraction k-block kt), or None for a
# zero-padded slot.
ASSIGN = [
    ((0, 3, 0), (0, 3, 1), (0, 3, 2)),
    ((0, 3, 3), (1, 3, 1), (1, 3, 2)),
    ((1, 3, 3), (2, 3, 2), (2, 3, 3)),
    ((3, 3, 3), (0, 2, 0), (0, 2, 1)),
    ((0, 2, 2), (1, 2, 1), (1, 2, 2)),
    ((2, 2, 2), (0, 1, 0), (0, 1, 1)),
    ((1, 1, 1), (0, 0, 0), None),
    (None, None, None),
]
# C brick (R, nb) -> list of (core, slot) contributions to sum.
BRICK_SRC = {}
for _g, _slots in enumerate(ASSIGN):
    for _s, _u in enumerate(_slots):
        if _u is not None:
            BRICK_SRC.setdefault((_u[0], _u[1]), []).append((_g, _s))

_CACHE = {}


def _unpack_upper(p):
    """Packed row-major upper-tri -> dense [N, N] with zero lower triangle."""
    p = np.asarray(p, dtype=np.float32).reshape(-1)
    i = np.arange(N)[:, None]
    j = np.arange(N)[None, :]
    mask = j >= i
    pidx = np.where(mask, (i * (2 * N - i + 1)) // 2 + (j - i), 0)
    return np.where(mask, p[pidx], np.float32(0.0))


def _strip_framework_ceremony(nc):
    """IR surgery on the built program:
    - drop the 4 unused const-AP memsets in the entry block (they gate
      the entry all-engine barrier on the Pool engine by ~400ns);
    - drop the exit all-engine barrier EventSemaphores (the final SP
      wait_ge(osem) already guarantees the output landed; per-engine
      drains are kept);
    - hoist the three input DMACopies to the head of the entry block,
      ahead of SP's entry-barrier participation, so descriptor
      generation and the transfers overlap the barrier (~300ns).  Safe:
      nothing reads the SBUF tile before its per-chunk semaphore fires,
      and the runtime resets semaphores between executions (verified by
      repeat runs)."""
    import concourse.mybir as mybir

    f = nc.m.functions[0]
    entry = f.blocks[0]
    entry.instructions = [
        i
        for i in entry.instructions
        if not (
            isinstance(i, mybir.InstMemset)
            and i.outs
            and "const-" in str(getattr(i.outs[0].bass_ap.tensor, "name", ""))
        )
    ]
    for bb in f.blocks:
        if bb.name.endswith("_end"):
            bb.instructions = [
                i
                for i in bb.instructions
                if not (
                    isinstance(i, mybir.InstEventSemaphore)
                    and str(i.name).startswith("aeb_barrier")
                )
            ]
    moved = []
    for bb in f.blocks:
        dmas = [
            i
            for i in bb.instructions
            if isinstance(i, mybir.InstDMACopy)
            and i.outs
            and "t_" in str(getattr(i.outs[0].bass_ap.tensor, "name", ""))
        ]
        if dmas:
            bb.instructions = [i for i in bb.instructions if i not in dmas]
            moved += dmas
    entry.instructions = moved + entry.instructions


def _build_nc():
    import concourse.mybir as mybir
    from concourse import bacc

    F32 = mybir.dt.float32

    nc = bacc.Bacc("TRN2", num_devices=NCORES)
    ab = nc.dram_tensor("ab", [P, S, BW], F32, kind="ExternalInput")
    cdr = nc.dram_tensor("c", [P, S, 128], F32, kind="ExternalOutput")

    with (
        nc.sbuf_tensor([P, S, BW], F32) as t,
        nc.sbuf_tensor([P, S, 128], F32) as ostage,
        # One PSUM tensor per slot: independent accumulation groups, and
        # each slot's copy must not read another slot's open group.
        nc.psum_tensor([P, 128], F32) as ps0,
        nc.psum_tensor([P, 128], F32) as ps1,
        nc.psum_tensor([P, 128], F32) as ps2,
        # One semaphore per input chunk: DMAs sharing one sem could
        # interleave per-engine completions, so a cumulative wait wouldn't
        # prove an individual chunk landed (CoreSim race detector).
        nc.semaphore("ds0") as ds0,
        nc.semaphore("ds1") as ds1,
        nc.semaphore("ds2") as ds2,
        nc.semaphore("osem") as osem,
        nc.semaphore("osem_p") as osem_p,
        nc.semaphore("psem") as psem,
        nc.semaphore("vsem") as vsem,
        nc.Block(no_gpsimd_drain=True) as block,
    ):
        dsems = [ds0, ds1, ds2]
        psums = [ps0, ps1, ps2]

        # Slot 1's input chunk and output store ride the SWDGE (Pool)
        # path: Q7 descriptor generation runs in parallel with the HWDGE
        # chain, so chunk 1's transfer slots between chunks 0/2 (earlier
        # receipts for MM1/MM2) and out 1 stays off the HWDGE chain that
        # gates out 2.  SWDGE requires its semaphore to start from 0, so
        # the Pool store ticks its own osem_p.
        @block.sync
        def _(sync):
            for s in (0, 2):
                sync.dma_start(out=t.ap()[:, s], in_=ab.ap()[:, s]).then_inc(
                    dsems[s], 16
                )
            # Per-brick stores: slot s's output DMA launches as soon as its
            # copy lands, overlapping later bricks' matmuls/copies.
            for s in (0, 2):
                sync.wait_ge(vsem, s + 1)
                sync.dma_start(
                    out=cdr.ap()[:, s], in_=ostage.ap()[:, s]
                ).then_inc(osem, 16)
            # Wait on the later-completing Pool store first so the
            # already-satisfied HWDGE wait costs no extra sequencer time.
            sync.wait_ge(osem_p, 16)
            sync.wait_ge(osem, 32)

        @block.gpsimd
        def _(gp):
            gp.dma_start(out=t.ap()[:, 1], in_=ab.ap()[:, 1]).then_inc(
                dsems[1], 16
            )
            gp.wait_ge(vsem, 2)
            gp.dma_start(out=cdr.ap()[:, 1], in_=ostage.ap()[:, 1]).then_inc(
                osem_p, 16
            )

        @block.tensor
        def _(tensor):
            for s in range(S):
                tensor.wait_ge(dsems[s], 16)
                nc.tensor.matmul(
                    psums[s].ap(),
                    t.ap()[:, s, :128],
                    t.ap()[:, s, 128:],
                    start=True,
                    stop=True,
                ).then_inc(psem, 1)

        @block.vector
        def _(vector):
            for s in range(S):
                vector.wait_ge(psem, s + 1)
                nc.vector.tensor_copy(
                    ostage.ap()[:, s], psums[s].ap()
                ).then_inc(vsem, 1)

    _strip_framework_ceremony(nc)
    nc.compile()
    return nc


def _get_nc():
    if "nc" not in _CACHE:
        _CACHE["nc"] = _build_nc()
    return _CACHE["nc"]


def _make_in_maps(A, B):
    Au = _unpack_upper(A)
    Bu = _unpack_upper(B)
    aT = np.ascontiguousarray(Au.T)  # aT[k, m] = Au[m, k]
    aTk = aT.reshape(KT, P, N)  # [kt, p, m]
    Buk = Bu.reshape(KT, P, N)  # [kt, p, n]
    in_maps = []
    for slots in ASSIGN:
        abarr = np.zeros((P, S, BW), dtype=np.float32)
        for s, unit in enumerate(slots):
            if unit is None:
                continue
            R, nb, kt = unit
            abarr[:, s, :128] = aTk[kt, :, R * P : (R + 1) * P]
            abarr[:, s, 128:] = Buk[kt, :, nb * P : (nb + 1) * P]
        in_maps.append({"ab": abarr})
    return in_maps


def _get_runner():
    """Build the sharded PJRT executable once; reuse across kernel() calls.

    Mirrors concourse.bass2jax.run_bass_via_pjrt's multi-core path, but
    caches the jitted function so repeat calls skip retracing.
    """
    if "runner" in _CACHE:
        return _CACHE["runner"]
    import jax
    import concourse.mybir as mybir
    from concourse import bass2jax
    from jax.experimental.shard_map import shard_map
    from jax.sharding import Mesh, PartitionSpec

    nc = _get_nc()
    bass2jax.install_neuronx_cc_hook()
    partition_name = (
        nc.partition_id_tensor.name if nc.partition_id_tensor else None
    )
    in_names, out_names, out_avals, zero_outs = [], [], [], []
    for alloc in nc.m.functions[0].allocations:
        if not isinstance(alloc, mybir.MemoryLocationSet):
            continue
        name = alloc.memorylocations[0].name
        if alloc.kind == "ExternalInput":
            if name != partition_name:
                in_names.append(name)
        elif alloc.kind == "ExternalOutput":
            out_names.append(name)
            shape = tuple(alloc.tensor_shape)
            dtype = mybir.dt.np(alloc.dtype)
            out_avals.append(jax.core.ShapedArray(shape, dtype))
            zero_outs.append(np.zeros(shape, dtype))
    n_params = len(in_names)
    n_outs = len(out_names)
    all_in = in_names + out_names + ([partition_name] if partition_name else [])
    donate = tuple(range(n_params, n_params + n_outs))

    def _body(*args):
        operands = list(args)
        if partition_name is not None:
            operands.append(bass2jax.partition_id_tensor())
        outs = bass2jax._bass_exec_p.bind(
            *operands,
            out_avals=tuple(out_avals),
            in_names=tuple(all_in),
            out_names=tuple(out_names),
            lowering_input_output_aliases=(),
            sim_require_finite=True,
            sim_require_nnan=True,
            nc=nc,
        )
        return tuple(outs)

    devices = jax.devices()[:NCORES]
    mesh = Mesh(np.asarray(devices), ("core",))
    fn = jax.jit(
        shard_map(
            _body,
            mesh=mesh,
            in_specs=(PartitionSpec("core"),) * (n_params + n_outs),
            out_specs=(PartitionSpec("core"),) * n_outs,
            check_rep=False,
        ),
        donate_argnums=donate,
        keep_unused=True,
    )
    runner = dict(
        fn=fn, in_names=in_names, out_names=out_names, zero_outs=zero_outs
    )
    _CACHE["runner"] = runner
    return runner


def _run_concat(concat_in):
    """Execute on 8 cores given axis-0-concatenated per-core inputs."""
    r = _get_runner()
    concat_zeros = [
        np.zeros((NCORES * z.shape[0], *z.shape[1:]), z.dtype)
        for z in r["zero_outs"]
    ]
    return r["fn"](*concat_in, *concat_zeros)


def _concat_inputs(in_maps):
    r = _get_runner()
    return [
        np.concatenate([in_maps[c][n] for c in range(NCORES)], axis=0)
        for n in r["in_names"]
    ]


def _assemble(out0):
    # out0: concat over cores of [P, S, 128] -> [NCORES, P(m), S, 128(n)]
    bricks = np.asarray(out0).reshape(NCORES, P, S, 128)
    C = np.zeros((N, N), dtype=np.float32)
    for (R, nb), srcs in BRICK_SRC.items():
        (g0, s0) = srcs[0]
        acc = bricks[g0, :, s0, :].copy()
        for g, s in srcs[1:]:
            acc += bricks[g, :, s, :]
        C[R * P : (R + 1) * P, nb * P : (nb + 1) * P] = acc
    return C


def kernel(A, B):
    in_maps = _make_in_maps(A, B)
    concat_in = _concat_inputs(in_maps)
    out = _run_concat(concat_in)
    return _assemble(out[0])
